# revision 1
# baseline (speedup 1.0000x reference)
"""Multi-head causal attention (B=1, S=4096, D=1024, H=16, HD=64) on 8
Trainium2 NeuronCores.

Sharding: head-parallel — 16 heads / 8 cores = 2 heads per core (one
128-channel slice of the QKV/output projections per core).

Per-core algorithm (all layouts transposed so the contraction dim sits on
SBUF partitions and softmax exp reads PSUM directly):
  phase 1  QKV projections from pre-transposed xT [D, S]:
             qT, kT [128, 4096] (d-contract matmuls, outputs transposed)
             V natural [4096, 128] via one PE transpose per 128-tile,
             stored per j-tile as [V_A | ones | V_B]
  phase 2  flash-style attention, no max-subtraction (scores ~ N(0,1)):
             scoresT psum [j, A|B] = kT_j.T @ qT_q (2 heads packed via
             tile_position row strips, K=64 each, one shared psum tile so
             the pair issues back-to-back)
             PT = exp(scoresT/8)  (ScalarE, reads PSUM, writes SBUF f32r)
             causal: strictly-upper j-blocks skipped, partial-width moving
             operands on diagonal blocks, one [128,128] triangle mask
             multiply per diagonal strip (GpSimd)
             acc[128, q] += [V_j | 1*64].T @ PT_j  (M=128: rows 64:128 =
             softmax denominator l replicated across partitions for free)
             normalize: attnT = acc_out * exp(-ln(l)) (ScalarE recip — Exp
             and Ln pinned to one ACT table set; DVE multiply)
  phase 3  output projection partial: partialT[o, s] = WoT_c.T @ attnT,
             written transposed [1024, 4096] per core.

Host: sums the 8 partials and transposes back to [1, S, D].

Matmuls run in float32r (TF32-like, ~1.5e-4 rel err per matmul, 1 cyc/row
at N>=256 vs 4 cyc/row for plain fp32; bf16 measured only ~5% faster).
"""

import os
import sys

import numpy as np

for _p in ("/opt/trn_rl_repo", "/root/.axon_site/_ro/trn_rl_repo"):
    if os.path.isdir(_p) and _p not in sys.path:
        sys.path.insert(0, _p)

from contextlib import ExitStack

import concourse.bass as bass
import concourse.tile as tile
from concourse import bacc, bass_utils, mybir
from concourse.masks import make_identity
import concourse.hw_specs as _hw_specs
import functools as _functools

# Pin Exp/Ln to the one table set containing both, so the softmax exp and
# the exp(-ln(l)) normalization never ping-pong ACT_TABLE_LOADs. Only the
# *selection* map is filtered; set order (= act_func_set_id) is unchanged.
_orig_get_tables = _hw_specs.get_activation_tables


@_functools.cache
def _pinned_tables(arch):
    t = dict(_orig_get_tables(arch))
    strip = {mybir.ActivationFunctionType.Exp, mybir.ActivationFunctionType.Ln}
    for name in t:
        if name != "natural_log_exp_and_others":
            t[name] = t[name] - strip
    return t


_hw_specs.get_activation_tables = _pinned_tables
bacc.get_activation_tables = _pinned_tables

# Problem shape (hardcoded per the harness contract).
B, S, D, H = 1, 4096, 1024, 16
HD = D // H          # 64
NCORES = 8
HPC = H // NCORES    # 2 heads per core
M = HPC * HD         # 128 channels per core
SBK = 512            # s/q block size
NSB = S // SBK       # 8
DBK = 128            # d block size
NDB = D // DBK       # 8
JBK = 128            # j (key) block size
VW = 3 * HD          # v_aug row width per j-tile: [V_A | V_B | ones]

F32 = mybir.dt.float32
F32R = mybir.dt.float32r

_CACHE = {}


def _build_nc():
    """Build + compile the per-core Bass program (identical on all cores)."""
    nc = bacc.Bacc("TRN2", target_bir_lowering=False, debug=False,
                   num_devices=NCORES)

    xT = nc.dram_tensor("xT", [D, S], F32R, kind="ExternalInput").ap()
    wq = nc.dram_tensor("wq", [D, M], F32R, kind="ExternalInput").ap()
    wk = nc.dram_tensor("wk", [D, M], F32R, kind="ExternalInput").ap()
    wv = nc.dram_tensor("wv", [D, M], F32R, kind="ExternalInput").ap()
    wo = nc.dram_tensor("wo", [M, D], F32R, kind="ExternalInput").ap()
    ones = nc.dram_tensor("ones", [128, HD], F32R, kind="ExternalInput").ap()
    dmask = nc.dram_tensor("dmask", [JBK, JBK], F32R,
                           kind="ExternalInput").ap()
    outp = nc.dram_tensor("outp", [D, S], F32, kind="ExternalOutput").ap()

    with tile.TileContext(nc) as tc:
        with ExitStack() as ctx:
            _emit(ctx, tc, nc, xT, wq, wk, wv, wo, ones, dmask, outp)
    nc.compile()
    return nc


def _emit(ctx, tc, nc, xT, wq, wk, wv, wo, ones, dmask, outp):
    const = ctx.enter_context(tc.tile_pool(name="const", bufs=1))
    persist = ctx.enter_context(tc.tile_pool(name="persist", bufs=1))
    xt_pool = ctx.enter_context(tc.tile_pool(name="xt", bufs=8))
    vtmp_pool = ctx.enter_context(tc.tile_pool(name="vtmp", bufs=3))
    pt_pool = ctx.enter_context(tc.tile_pool(name="pt", bufs=8))
    out_pool = ctx.enter_context(tc.tile_pool(name="outt", bufs=4))
    small = ctx.enter_context(tc.tile_pool(name="small", bufs=4))
    ps6k = ctx.enter_context(tc.tile_pool(name="ps6k", bufs=2, space="PSUM"))
    ps2k = ctx.enter_context(tc.tile_pool(name="ps2k", bufs=4, space="PSUM"))

    # ---- constants / persistent SBUF ----
    ident = const.tile([128, 128], F32)
    make_identity(nc, ident)

    wq_sb = const.tile([128, D], F32R)   # 8 d-tiles side by side [d, m]
    wk_sb = const.tile([128, D], F32R)
    wv_sb = const.tile([128, D], F32R)
    wo_sb = const.tile([128, D], F32R)   # [m, o]
    def _w_chunk(w_sb, w_dram, c, n=2):
        w_r = w_dram.rearrange("(d p) m -> p d m", p=DBK)
        w_sb_r = w_sb[:].rearrange("p (d m) -> p d m", d=NDB)
        nc.sync.dma_start(out=w_sb_r[:, c:c + n, :], in_=w_r[:, c:c + n, :])

    wtrips = ((wq_sb, wq), (wk_sb, wk), (wv_sb, wv))
    for c in (0, 2, 4, 6):
        for w_sb, w_dram in wtrips:
            _w_chunk(w_sb, w_dram, c, 2)

    def _late_consts():
        for c in range(4):
            nc.sync.dma_start(out=wo_sb[:, bass.ts(c, 256)],
                              in_=wo[:, bass.ts(c, 256)])
        nc.sync.dma_start(out=mask_sb[:], in_=dmask[:])

    mask_sb = const.tile([128, JBK], F32R)

    qT_sb = persist.tile([128, S], F32R)
    kT_sb = persist.tile([128, S], F32R)
    NJT = S // JBK   # 32 j-tiles
    v_aug = persist.tile([128, NJT * VW], F32R)
    attnT = persist.tile([128, S], F32R)

    # per j-tile layout [V_A | ones | V_B]: head A reads cols 0:128
    # ([V_A | 1] -> acc rows 64:128 = denominator l), head B reads cols
    # 64:192 ([1 | V_B] -> acc rows 0:64 = l). ones filled by one
    # broadcast DMA (step-0 middle dim on input).
    v_aug_r = v_aug[:].rearrange("p (t c w) -> p t c w", c=3, w=HD)
    ones_bcast = bass.AP(
        tensor=ones.tensor, offset=0,
        ap=[[HD, 128], [0, NJT], [1, HD]])
    nc.sync.dma_start(out=v_aug_r[:, :, 1, :], in_=ones_bcast)

    def phase1(sb):
        """QKV projections for s-block sb (512 rows of the sequence)."""
        q_ps = ps6k.tile([128, SBK], F32, tag="sc")
        k_ps = ps6k.tile([128, SBK], F32, tag="sc")
        vT_ps = ps2k.tile([128, SBK], F32, tag="small")
        for d in range(NDB):
            xt = xt_pool.tile([128, SBK], F32R, tag="xt")
            if sb == 0:
                half = SBK // 2
                for h in range(2):
                    nc.gpsimd.dma_start(
                        out=xt[:, h * half:(h + 1) * half],
                        in_=xT[bass.ts(d, DBK),
                               sb * SBK + h * half:sb * SBK + (h + 1) * half])
            else:
                nc.gpsimd.dma_start(out=xt[:],
                                    in_=xT[bass.ts(d, DBK), bass.ts(sb, SBK)])
            st, sp = d == 0, d == NDB - 1
            nc.tensor.matmul(q_ps[:], lhsT=wq_sb[:, bass.ts(d, M)],
                             rhs=xt[:], start=st, stop=sp)
            nc.tensor.matmul(k_ps[:], lhsT=wk_sb[:, bass.ts(d, M)],
                             rhs=xt[:], start=st, stop=sp)
            nc.tensor.matmul(vT_ps[:], lhsT=wv_sb[:, bass.ts(d, M)],
                             rhs=xt[:], start=st, stop=sp)
        nc.vector.tensor_copy(qT_sb[:, bass.ts(sb, SBK)], q_ps[:])
        nc.vector.tensor_copy(kT_sb[:, bass.ts(sb, SBK)], k_ps[:])
        vt = vtmp_pool.tile([128, SBK], F32)
        nc.vector.tensor_copy(vt[:], vT_ps[:])
        # vT [m, s] -> V natural [s, m] per 128-tile, into v_aug slots
        for t in range(SBK // JBK):
            jt = sb * (SBK // JBK) + t     # global j-tile index
            tp_ps = ps2k.tile([128, 128], F32, tag="small")
            nc.tensor.transpose(tp_ps[:], vt[:, bass.ts(t, 128)], ident[:])
            nc.vector.tensor_copy(v_aug_r[:, jt, 0::2, :], tp_ps[:, 0:2 * HD])

    def attention(qb):
        """Causal attention for query block qb (both heads)."""
        nj = 4 * (qb + 1)               # valid j128-blocks
        acc_A = ps2k.tile([128, SBK], F32, tag="small")
        acc_B = ps2k.tile([128, SBK], F32, tag="small")
        qsl = bass.ts(qb, SBK)
        for j in range(nj):
            # diagonal j-block r: columns q < 128*r are fully masked ->
            # process only [off:SBK] (partial moving operand).
            # sc/pt hold BOTH heads ([A | B]) so the row-packed score
            # matmul pair shares one slot release and issues back-to-back.
            r = j - (nj - 4)
            off = 128 * r if r > 0 else 0
            sc = ps6k.tile([128, 2 * SBK], F32, tag="sc")
            qa = qT_sb[0:64, qb * SBK + off:(qb + 1) * SBK]
            qb_ap = qT_sb[64:128, qb * SBK + off:(qb + 1) * SBK]
            nc.tensor.matmul(sc[:, off:SBK],
                             lhsT=kT_sb[0:64, bass.ts(j, JBK)],
                             rhs=qa, start=True, stop=True)
            nc.tensor.matmul(sc[:, SBK + off:2 * SBK],
                             lhsT=kT_sb[64:128, bass.ts(j, JBK)],
                             rhs=qb_ap, start=True, stop=True)
            pt = pt_pool.tile([128, 2 * SBK], F32R, tag="pt")
            scale = float(1.0 / np.sqrt(HD))
            if off == 0:
                nc.scalar.activation(pt[:], sc[:],
                                     mybir.ActivationFunctionType.Exp,
                                     scale=scale)
            else:
                w = SBK - off
                sc2 = bass.AP(tensor=sc.tensor, offset=sc.offset + off,
                              ap=[list(sc.ap[0]), [SBK, 2], [1, w]])
                pt2 = bass.AP(tensor=pt.tensor, offset=pt.offset + off,
                              ap=[list(pt.ap[0]), [SBK, 2], [1, w]])
                nc.scalar.activation(pt2, sc2,
                                     mybir.ActivationFunctionType.Exp,
                                     scale=scale)
            if r >= 0:
                # triangle mask on the [128,128] diagonal strip, per head
                dlo = 128 * r
                nc.gpsimd.tensor_mul(pt[:, dlo:dlo + 128],
                                     pt[:, dlo:dlo + 128], mask_sb[:])
                nc.gpsimd.tensor_mul(pt[:, SBK + dlo:SBK + dlo + 128],
                                     pt[:, SBK + dlo:SBK + dlo + 128],
                                     mask_sb[:])
            st, sp = j == 0, j == nj - 1
            vb = j * VW
            nc.tensor.matmul(acc_A[:, off:SBK],
                             lhsT=v_aug[:, vb:vb + 128],
                             rhs=pt[:, off:SBK], start=st, stop=sp)
            nc.tensor.matmul(acc_B[:, off:SBK],
                             lhsT=v_aug[:, vb + HD:vb + VW],
                             rhs=pt[:, SBK + off:2 * SBK],
                             start=st, stop=sp)
        return acc_A, acc_B

    def normalize(qb, acc_A, acc_B):
        # head A: out rows 0:64, l rows 64:128; head B flipped
        qsl = bass.ts(qb, SBK)
        # 1/l as exp(-ln(l)) on ScalarE (natural_log_exp_and_others set
        # holds both funcs, so no table switches).
        lnl = small.tile([64, SBK], F32, tag="lnl")
        nc.scalar.activation(lnl[:], acc_A[HD:2 * HD, :],
                             mybir.ActivationFunctionType.Ln)
        linv = small.tile([64, SBK], F32, tag="linv")
        nc.scalar.activation(linv[:], lnl[:],
                             mybir.ActivationFunctionType.Exp, scale=-1.0)
        nc.vector.tensor_mul(attnT[0:64, qsl], acc_A[0:HD, :], linv[:])
        lnl_b = small.tile([64, SBK], F32, tag="lnl")
        nc.scalar.activation(lnl_b[:], acc_B[0:HD, :],
                             mybir.ActivationFunctionType.Ln)
        linv_b = small.tile([64, SBK], F32, tag="linv")
        nc.scalar.activation(linv_b[:], lnl_b[:],
                             mybir.ActivationFunctionType.Exp, scale=-1.0)
        nc.vector.tensor_mul(attnT[64:128, qsl], acc_B[HD:2 * HD, :],
                             linv_b[:])

    def proj(qb):
        """Output-projection partial for s-block qb -> DRAM (transposed)."""
        qsl = bass.ts(qb, SBK)
        for ob in range(NDB):
            po = ps2k.tile([128, SBK], F32, tag="small")
            nc.tensor.matmul(po[:], lhsT=wo_sb[:, bass.ts(ob, 128)],
                             rhs=attnT[:, qsl], start=True, stop=True)
            ot = out_pool.tile([128, SBK], F32)
            nc.vector.tensor_copy(ot[:], po[:])
            nc.sync.dma_start(out=outp[bass.ts(ob, 128), qsl], in_=ot[:])

    # interleaved emission: attention(qb) only needs kT/v for s-blocks <= qb.
    # phase1(sb) is emitted before normalize(qb-1) so its PSUM evictions are
    # not queued on DVE behind the end-of-block normalization.
    phase1(0)
    _late_consts()
    for sb in range(1, NSB):
        accs = attention(sb - 1)
        normalize(sb - 1, *accs)
        phase1(sb)
        proj(sb - 1)
    accs = attention(NSB - 1)
    normalize(NSB - 1, *accs)
    proj(NSB - 1)


def _host_prep(x, Wq, Wk, Wv, Wo):
    xT = np.ascontiguousarray(x.reshape(S, D).T).astype(np.float32)
    jj = np.arange(JBK)[:, None]
    qq = np.arange(JBK)[None, :]
    dmask = (jj <= qq).astype(np.float32)
    in_maps = []
    for c in range(NCORES):
        sl = slice(c * M, (c + 1) * M)
        in_maps.append({
            "xT": xT,
            "wq": np.ascontiguousarray(Wq[sl, :].T).astype(np.float32),
            "wk": np.ascontiguousarray(Wk[sl, :].T).astype(np.float32),
            "wv": np.ascontiguousarray(Wv[sl, :].T).astype(np.float32),
            "wo": np.ascontiguousarray(Wo[:, sl].T).astype(np.float32),
            "ones": np.ones((128, HD), dtype=np.float32),
            "dmask": dmask,
        })
    return in_maps


def _run(inputs, trace=False):
    x = np.asarray(inputs["x"], dtype=np.float32)
    Wq = np.asarray(inputs["Wq"], dtype=np.float32)
    Wk = np.asarray(inputs["Wk"], dtype=np.float32)
    Wv = np.asarray(inputs["Wv"], dtype=np.float32)
    Wo = np.asarray(inputs["Wo"], dtype=np.float32)

    if "nc" not in _CACHE:
        _CACHE["nc"] = _build_nc()
    nc = _CACHE["nc"]

    in_maps = _host_prep(x, Wq, Wk, Wv, Wo)
    res = bass_utils.run_bass_kernel_spmd(
        nc, in_maps, core_ids=list(range(NCORES)), trace=trace)
    partial = np.zeros((D, S), dtype=np.float64)
    for c in range(NCORES):
        partial += res.results[c]["outp"].astype(np.float64)
    out = partial.T.astype(np.float32).reshape(B, S, D)
    return out, res


def kernel(x, mask, Wq, Wk, Wv, Wo):
    mask = np.asarray(mask)
    causal = np.tril(np.ones((S, S), dtype=bool))
    if mask.reshape(S, S).shape == causal.shape and bool(
            np.array_equal(mask.reshape(S, S), causal)):
        out, _ = _run({"x": x, "Wq": Wq, "Wk": Wk, "Wv": Wv, "Wo": Wo})
        return out
    # safety net for a non-causal mask: exact numpy fallback
    return _numpy_ref(np.asarray(x, np.float32), mask,
                      np.asarray(Wq, np.float32), np.asarray(Wk, np.float32),
                      np.asarray(Wv, np.float32), np.asarray(Wo, np.float32))


def _numpy_ref(x, mask, Wq, Wk, Wv, Wo):
    xf = x.reshape(S, D)
    q = xf @ Wq.T
    k = xf @ Wk.T
    v = xf @ Wv.T
    m2 = mask.reshape(S, S)
    o = np.empty((S, D), dtype=np.float32)
    for h in range(H):
        hs = slice(h * HD, (h + 1) * HD)
        sc = (q[:, hs] @ k[:, hs].T) / np.sqrt(np.float32(HD))
        sc = np.where(m2, sc, np.float32(-1e9))
        sc -= sc.max(axis=-1, keepdims=True)
        p = np.exp(sc)
        p /= p.sum(axis=-1, keepdims=True)
        o[:, hs] = p @ v[:, hs]
    return (o @ Wo.T).astype(np.float32).reshape(B, S, D)



# revision 23
# speedup vs baseline: 1.0650x; 1.0650x over previous
"""Multi-head causal attention (B=1, S=4096, D=1024, H=16, HD=64) on 8
Trainium2 NeuronCores.

Sharding: head-parallel - 16 heads / 8 cores = 2 heads per core (one
128-channel slice of the QKV/output projections per core).

Per-core pipeline (contraction dims on SBUF partitions; softmax exp
reads PSUM directly; all attention matmuls in bf16 - fp8 was measured
to put ~5% relative error on the output because softmax-weight noise
is not damped by the value average):

  phase 1 (interleaved into the attention j-loop as filler PE units):
    qT/kT [128, S] bf16 via wq-stationary matmuls (outputs transposed),
    V via vT matmuls + PE transpose per 128-tile, stored per j-tile as
    [V_A | ones | V_B] in bf16.
  phase 2 flash-style attention, no max-subtraction (scores ~ N(0,1)):
    scoresT psum[j, A|B] = kT_j.T @ qT_q (2 heads, K=64 each)
    PT = exp(scores/8): ScalarE reads PSUM, writes bf16 SBUF
    causal: strictly-upper j-blocks skipped, partial-width operands on
      diagonal blocks, one [128,128] triangle mask multiply (GpSimd);
      diagonal blocks processed FIRST so their mask latency hides
      under the long run of full-width blocks.
    acc += [V | ones].T @ PT (bf16, rows 64:128 = softmax denominator)
    The j-loop is software-pipelined: scores(j+1) are emitted before
      attnV(j), so the PE never head-of-line blocks on the exp.
  normalize: 1/l via DVE reciprocal (keeps ScalarE exp-only),
    attnT = acc * (1/l) on DVE.
  phase 3 output projection partial (f32r): partialT[o, s], written
    transposed [1024, 4096] per core; host sums the 8 partials.
"""

import os
import sys

import numpy as np

for _p in ("/opt/trn_rl_repo", "/root/.axon_site/_ro/trn_rl_repo"):
    if os.path.isdir(_p) and _p not in sys.path:
        sys.path.insert(0, _p)

from contextlib import ExitStack

import ml_dtypes

import concourse.bass as bass
import concourse.tile as tile
from concourse import bacc, bass_utils, mybir
from concourse.masks import make_identity

# Problem shape (hardcoded per the harness contract).
B, S, D, H = 1, 4096, 1024, 16
HD = D // H          # 64
NCORES = 8
HPC = H // NCORES    # 2 heads per core
M = HPC * HD         # 128 channels per core
SBK = 512            # s/q block size
NSB = S // SBK       # 8
JBK = 128            # j (key) block size
NJT = S // JBK       # 32
VW = 3 * HD          # v_aug row width per j-tile: [V_A | ones | V_B]

F32 = mybir.dt.float32
F32R = mybir.dt.float32r
BF16 = mybir.dt.bfloat16

NP_BF16 = ml_dtypes.bfloat16

_CACHE = {}


def _build_nc():
    """Build + compile the per-core Bass program (identical on all cores)."""
    nc = bacc.Bacc("TRN2", target_bir_lowering=False, debug=False,
                   num_devices=NCORES)

    # bf16 x, pre-transposed: [p, dt, s] with d = dt*128+p
    xb = nc.dram_tensor("xb", [128, 8 * S], BF16, kind="ExternalInput").ap()
    # bf16 weights, [p, dt, m] = W[m_local, dt*128+p]
    wq = nc.dram_tensor("wq", [128, 8 * M], BF16, kind="ExternalInput").ap()
    wk = nc.dram_tensor("wk", [128, 8 * M], BF16, kind="ExternalInput").ap()
    wv = nc.dram_tensor("wv", [128, 8 * M], BF16, kind="ExternalInput").ap()
    wo = nc.dram_tensor("wo", [M, D], F32R, kind="ExternalInput").ap()
    ones = nc.dram_tensor("ones", [128, HD], BF16, kind="ExternalInput").ap()
    dmask = nc.dram_tensor("dmask", [JBK, JBK], BF16,
                           kind="ExternalInput").ap()
    outp = nc.dram_tensor("outp", [D, S], F32, kind="ExternalOutput").ap()

    with tile.TileContext(nc) as tc:
        with ExitStack() as ctx:
            _emit(ctx, tc, nc, xb, wq, wk, wv, wo, ones, dmask, outp)
    nc.compile()
    return nc


def _emit(ctx, tc, nc, xb, wq, wk, wv, wo, ones, dmask, outp):
    const = ctx.enter_context(tc.tile_pool(name="const", bufs=1))
    persist = ctx.enter_context(tc.tile_pool(name="persist", bufs=1))
    xb_pool = ctx.enter_context(tc.tile_pool(name="xb", bufs=3))
    vt_pool = ctx.enter_context(tc.tile_pool(name="vt", bufs=2))
    pt_pool = ctx.enter_context(tc.tile_pool(name="pt", bufs=4))
    ot_pool = ctx.enter_context(tc.tile_pool(name="ot", bufs=8))
    lv_pool = ctx.enter_context(tc.tile_pool(name="lv", bufs=2))
    # PSUM: 16 KB/partition = 8 banks. sc 2x(2 banks) + acc 2x1 + work 2x1.
    ps_sc = ctx.enter_context(tc.tile_pool(name="ps_sc", bufs=2, space="PSUM"))
    ps_acc = ctx.enter_context(tc.tile_pool(name="ps_acc", bufs=2,
                                            space="PSUM"))
    ps_wk = ctx.enter_context(tc.tile_pool(name="ps_wk", bufs=2, space="PSUM"))

    # ---- constants ----
    ident = const.tile([128, 128], F32)
    make_identity(nc, ident)
    wq_sb = const.tile([128, 8 * M], BF16)
    wk_sb = const.tile([128, 8 * M], BF16)
    wv_sb = const.tile([128, 8 * M], BF16)
    wo_sb = const.tile([128, D], F32R)
    mask_sb = const.tile([128, JBK], BF16)
    nc.sync.dma_start(out=wq_sb[:], in_=wq[:])
    nc.sync.dma_start(out=wk_sb[:], in_=wk[:])
    nc.sync.dma_start(out=wv_sb[:], in_=wv[:])
    nc.sync.dma_start(out=mask_sb[:], in_=dmask[:])

    def _late_consts():
        for c in range(4):
            nc.sync.dma_start(out=wo_sb[:, bass.ts(c, 256)],
                              in_=wo[:, bass.ts(c, 256)])

    wq_r = wq_sb[:].rearrange("p (dt m) -> p dt m", dt=8)
    wk_r = wk_sb[:].rearrange("p (dt m) -> p dt m", dt=8)
    wv_r = wv_sb[:].rearrange("p (dt m) -> p dt m", dt=8)

    # ---- persistent SBUF ----
    qT_sb = persist.tile([128, S], BF16)
    kT_sb = persist.tile([128, S], BF16)
    v_aug = persist.tile([128, NJT * VW], BF16)
    attnT = persist.tile([128, S], F32R)

    # per j-tile layout [V_A | ones | V_B]: head A reads cols 0:128
    # ([V_A | 1] -> acc rows 64:128 = denominator l), head B reads cols
    # 64:192 ([1 | V_B] -> acc rows 0:64 = l). ones filled by one
    # broadcast DMA (step-0 middle dim on input).
    v_aug_r = v_aug[:].rearrange("p (t c w) -> p t c w", c=3, w=HD)
    ones_bcast = bass.AP(
        tensor=ones.tensor, offset=0,
        ap=[[HD, 128], [0, NJT], [1, HD]])
    nc.sync.dma_start(out=v_aug_r[:, :, 1, :], in_=ones_bcast)

    xb_r_dram = xb.rearrange("p (dt s) -> p dt s", dt=8)

    def phase1_dma(sb):
        """x loads for s-block sb (issued one attention block early)."""
        xbt = xb_pool.tile([128, 8 * SBK], BF16, tag="xb", name="xbt")
        xb_r = xbt[:].rearrange("p (dt s) -> p dt s", dt=8)
        sl = slice(sb * SBK, (sb + 1) * SBK)
        if sb == 0:
            # dtile-major chunks so the first q matmuls start after ~1
            # chunk instead of the full 8 KB/partition load.
            for d0 in range(0, 8, 2):
                nc.gpsimd.dma_start(out=xb_r[:, d0:d0 + 2, :],
                                    in_=xb_r_dram[:, d0:d0 + 2, sl])
        else:
            nc.gpsimd.dma_start(out=xb_r[:, :4, :], in_=xb_r_dram[:, :4, sl])
            nc.gpsimd.dma_start(out=xb_r[:, 4:, :], in_=xb_r_dram[:, 4:, sl])
        return xb_r

    def phase1_units(sb, xb_r):
        """QKV projections for s-block sb, as PE work units."""
        units = []
        state = {}

        def qk_unit(which, lo):
            def run():
                w_r = {"q": wq_r, "k": wk_r, "v": wv_r}[which]
                if lo == 0:
                    state[which] = ps_wk.tile([128, SBK], F32, tag="work",
                                              name=f"ps_{which}")
                ps = state[which]
                for dt in range(lo, lo + 4):
                    nc.tensor.matmul(ps[:], lhsT=w_r[:, dt, :],
                                     rhs=xb_r[:, dt, :],
                                     start=dt == 0, stop=dt == 7)
                if lo == 4 and which != "v":
                    dst = qT_sb if which == "q" else kT_sb
                    nc.vector.tensor_copy(dst[:, bass.ts(sb, SBK)], ps[:])
            return run

        def v_evict():
            vt = vt_pool.tile([128, SBK], F32, tag="vt", name="vt")
            nc.vector.tensor_copy(vt[:], state["v"][:])
            state["vt"] = vt

        def t_unit():
            tp = ps_wk.tile([128, SBK], F32, tag="work", name="ps_tp")
            for t in range(4):
                jt = sb * 4 + t
                tsl = bass.ts(t, 128)
                nc.tensor.transpose(tp[:, tsl], state["vt"][:, tsl], ident[:])
                tp_r = bass.AP(tensor=tp.tensor, offset=tp.offset + t * 128,
                               ap=[list(tp.ap[0]), [HD, 2], [1, HD]])
                nc.vector.tensor_copy(v_aug_r[:, jt, 0::2, :], tp_r)

        for which in ("q", "k", "v"):
            for lo in (0, 4):
                units.append(qk_unit(which, lo))
        units.append(v_evict)
        units.append(t_unit)
        return units

    def proj_units(qb, tail=False):
        """Output-projection partial for s-block qb -> DRAM (transposed).
        At the tail there is no attention left to hide behind, so borrow
        the (now idle) score-psum banks and the ScalarE for every other
        eviction to shorten the drain."""
        qsl = bass.ts(qb, SBK)
        units = []

        def ob_unit(ob):
            def run():
                if tail and ob % 2 == 0:
                    big = ps_sc.tile([128, 2 * SBK], F32, tag="sc",
                                     name="ps_po_sc")
                    po = big[:, 0:SBK]
                else:
                    po = ps_wk.tile([128, SBK], F32, tag="work",
                                    name="ps_po")[:]
                nc.tensor.matmul(po, lhsT=wo_sb[:, bass.ts(ob, 128)],
                                 rhs=attnT[:, qsl], start=True, stop=True)
                ot = ot_pool.tile([128, SBK], F32, tag="ot")
                if tail and ob % 2 == 1:
                    nc.scalar.activation(ot[:], po,
                                         mybir.ActivationFunctionType.Copy)
                else:
                    nc.vector.tensor_copy(ot[:], po)
                if tail and ob % 3 == 1:
                    nc.gpsimd.dma_start(out=outp[bass.ts(ob, 128), qsl],
                                        in_=ot[:])
                elif tail and ob % 3 == 2:
                    nc.scalar.dma_start(out=outp[bass.ts(ob, 128), qsl],
                                        in_=ot[:])
                else:
                    nc.sync.dma_start(out=outp[bass.ts(ob, 128), qsl],
                                      in_=ot[:])
            return run

        for ob in range(8):
            units.append(ob_unit(ob))
        return units

    scale = float(1.0 / np.sqrt(HD))

    def attention(qb, units):
        """Causal attention for query block qb (both heads), with `units`
        (phase1/proj closures) interleaved into the PE stream."""
        nj = 4 * (qb + 1)               # valid j128-blocks
        # diagonal strips first: their GpSimd mask latency hides under
        # the long tail of full-width blocks.
        order = list(range(nj - 4, nj)) + list(range(0, nj - 4))
        offs = {j: max(0, 128 * (j - (nj - 4))) for j in order}

        acc_A = ps_acc.tile([128, SBK], F32, tag="acc", name="acc_A")
        acc_B = ps_acc.tile([128, SBK], F32, tag="acc", name="acc_B")

        def emit_sc(j):
            off = offs[j]
            sc = ps_sc.tile([128, 2 * SBK], F32, tag="sc", name="sc")
            qa = qT_sb[0:64, qb * SBK + off:(qb + 1) * SBK]
            qb_ap = qT_sb[64:128, qb * SBK + off:(qb + 1) * SBK]
            nc.tensor.matmul(sc[:, off:SBK],
                             lhsT=kT_sb[0:64, bass.ts(j, JBK)],
                             rhs=qa, start=True, stop=True)
            nc.tensor.matmul(sc[:, SBK + off:2 * SBK],
                             lhsT=kT_sb[64:128, bass.ts(j, JBK)],
                             rhs=qb_ap, start=True, stop=True)
            return sc

        def emit_exp(j, sc):
            off = offs[j]
            pt = pt_pool.tile([128, 2 * SBK], BF16, tag="pt", name="pt")
            if off == 0:
                nc.scalar.activation(pt[:], sc[:],
                                     mybir.ActivationFunctionType.Exp,
                                     scale=scale)
            else:
                w = SBK - off
                sc2 = bass.AP(tensor=sc.tensor, offset=sc.offset + off,
                              ap=[list(sc.ap[0]), [SBK, 2], [1, w]])
                pt2 = bass.AP(tensor=pt.tensor, offset=pt.offset + off,
                              ap=[list(pt.ap[0]), [SBK, 2], [1, w]])
                nc.scalar.activation(pt2, sc2,
                                     mybir.ActivationFunctionType.Exp,
                                     scale=scale)
            return pt

        def emit_mask(j, pt):
            r = j - (nj - 4)
            dlo = 128 * r
            nc.gpsimd.tensor_mul(pt[:, dlo:dlo + 128],
                                 pt[:, dlo:dlo + 128], mask_sb[:])
            nc.gpsimd.tensor_mul(pt[:, SBK + dlo:SBK + dlo + 128],
                                 pt[:, SBK + dlo:SBK + dlo + 128],
                                 mask_sb[:])

        def emit_pv(idx, j, pt):
            off = offs[j]
            st, sp = idx == 0, idx == nj - 1
            vb = j * VW
            nc.tensor.matmul(acc_A[:, off:SBK],
                             lhsT=v_aug[:, vb:vb + 128],
                             rhs=pt[:, off:SBK], start=st, stop=sp)
            nc.tensor.matmul(acc_B[:, off:SBK],
                             lhsT=v_aug[:, vb + HD:vb + VW],
                             rhs=pt[:, SBK + off:2 * SBK],
                             start=st, stop=sp)

        emitted = 0
        sc = emit_sc(order[0])
        for idx, j in enumerate(order):
            pt = emit_exp(j, sc)
            if j >= nj - 4:
                emit_mask(j, pt)
            if idx + 1 < nj:
                sc = emit_sc(order[idx + 1])
            # filler units go BEFORE pv(j): pv waits on the exp semaphore
            # anyway, so the PE chews through these instead of stalling.
            want = ((idx + 1) * len(units)) // nj
            while emitted < want:
                units[emitted]()
                emitted += 1
            emit_pv(idx, j, pt)
        return acc_A, acc_B

    def normalize(qb, acc_A, acc_B):
        # head A: out rows 0:64, l rows 64:128; head B flipped.
        # 1/l on DVE keeps ScalarE exp-only.
        qsl = bass.ts(qb, SBK)
        linv_a = lv_pool.tile([64, SBK], F32, tag="lv")
        nc.vector.reciprocal(linv_a[:], acc_A[HD:2 * HD, :])
        nc.vector.tensor_mul(attnT[0:64, qsl], acc_A[0:HD, :], linv_a[:])
        linv_b = lv_pool.tile([64, SBK], F32, tag="lv")
        nc.vector.reciprocal(linv_b[:], acc_B[0:HD, :])
        nc.vector.tensor_mul(attnT[64:128, qsl], acc_B[HD:2 * HD, :],
                             linv_b[:])

    # prologue: phase1(0) un-interleaved, then the qb loop with
    # phase1(qb+1) + proj(qb-1) threaded into attention(qb)'s j-loop.
    # x loads are prefetched two blocks ahead so phase1 units never wait.
    xb_handles = {0: phase1_dma(0)}
    for u in phase1_units(0, xb_handles[0]):
        u()
    xb_handles[1] = phase1_dma(1)
    _late_consts()
    for qb in range(NSB):
        if qb + 2 < NSB:
            xb_handles[qb + 2] = phase1_dma(qb + 2)
        units = []
        if qb + 1 < NSB:
            units += phase1_units(qb + 1, xb_handles[qb + 1])
        if qb > 0:
            units += proj_units(qb - 1)
        accs = attention(qb, units)
        normalize(qb, *accs)
    for u in proj_units(NSB - 1, tail=True):
        u()


def _host_prep(x, Wq, Wk, Wv, Wo):
    xf = np.ascontiguousarray(x.reshape(S, D)).astype(np.float32)
    xT = xf.T  # [D, S]
    xb = np.ascontiguousarray(
        xT.reshape(8, 128, S).transpose(1, 0, 2)
    ).astype(NP_BF16).reshape(128, 8 * S)

    jj = np.arange(JBK)[:, None]
    qq = np.arange(JBK)[None, :]
    dmask = (jj <= qq).astype(NP_BF16)
    ones_arr = np.ones((128, HD), dtype=NP_BF16)

    def w_prep(W_local):
        # [p, dt, m] = W_local[m, dt*128+p]
        A = W_local.T.reshape(8, 128, M).transpose(1, 0, 2)
        return np.ascontiguousarray(A).astype(NP_BF16).reshape(128, -1)

    in_maps = []
    for c in range(NCORES):
        sl = slice(c * M, (c + 1) * M)
        in_maps.append({
            "xb": xb,
            "wq": w_prep(np.asarray(Wq[sl, :], np.float32)),
            "wk": w_prep(np.asarray(Wk[sl, :], np.float32)),
            "wv": w_prep(np.asarray(Wv[sl, :], np.float32)),
            "wo": np.ascontiguousarray(Wo[:, sl].T).astype(np.float32),
            "ones": ones_arr,
            "dmask": dmask,
        })
    return in_maps


def _run(inputs, trace=False):
    x = np.asarray(inputs["x"], dtype=np.float32)
    Wq = np.asarray(inputs["Wq"], dtype=np.float32)
    Wk = np.asarray(inputs["Wk"], dtype=np.float32)
    Wv = np.asarray(inputs["Wv"], dtype=np.float32)
    Wo = np.asarray(inputs["Wo"], dtype=np.float32)

    if "nc" not in _CACHE:
        _CACHE["nc"] = _build_nc()
    nc = _CACHE["nc"]

    in_maps = _host_prep(x, Wq, Wk, Wv, Wo)
    res = bass_utils.run_bass_kernel_spmd(
        nc, in_maps, core_ids=list(range(NCORES)), trace=trace)
    partial = np.zeros((D, S), dtype=np.float64)
    for c in range(NCORES):
        partial += res.results[c]["outp"].astype(np.float64)
    out = partial.T.astype(np.float32).reshape(B, S, D)
    return out, res


def kernel(x, mask, Wq, Wk, Wv, Wo):
    mask = np.asarray(mask)
    causal = np.tril(np.ones((S, S), dtype=bool))
    if mask.reshape(S, S).shape == causal.shape and bool(
            np.array_equal(mask.reshape(S, S), causal)):
        out, _ = _run({"x": x, "Wq": Wq, "Wk": Wk, "Wv": Wv, "Wo": Wo})
        return out
    # safety net for a non-causal mask: exact numpy fallback
    return _numpy_ref(np.asarray(x, np.float32), mask,
                      np.asarray(Wq, np.float32), np.asarray(Wk, np.float32),
                      np.asarray(Wv, np.float32), np.asarray(Wo, np.float32))


def _numpy_ref(x, mask, Wq, Wk, Wv, Wo):
    xf = x.reshape(S, D)
    q = xf @ Wq.T
    k = xf @ Wk.T
    v = xf @ Wv.T
    m2 = mask.reshape(S, S)
    o = np.empty((S, D), dtype=np.float32)
    for h in range(H):
        hs = slice(h * HD, (h + 1) * HD)
        sc = (q[:, hs] @ k[:, hs].T) / np.sqrt(np.float32(HD))
        sc = np.where(m2, sc, np.float32(-1e9))
        sc -= sc.max(axis=-1, keepdims=True)
        p = np.exp(sc)
        p /= p.sum(axis=-1, keepdims=True)
        o[:, hs] = p @ v[:, hs]
    return (o @ Wo.T).astype(np.float32).reshape(B, S, D)


# revision 27
# speedup vs baseline: 1.3300x; 1.2488x over previous
"""Multi-head causal attention (B=1, S=4096, D=1024, H=16, HD=64) on 8
Trainium2 NeuronCores.

Sharding: head-parallel - 16 heads / 8 cores = 2 heads per core (one
128-channel slice of the QKV/output projections per core).

Per-core pipeline (contraction dims on SBUF partitions; softmax exp
reads PSUM directly; all attention matmuls in bf16 - fp8 was measured
to put ~5% relative error on the output because softmax-weight noise
is not damped by the value average):

  phase 1 (interleaved into the attention j-loop as filler PE units):
    qT/kT [128, S] bf16 via wq-stationary matmuls (outputs transposed),
    V via vT matmuls + PE transpose per 128-tile, stored per j-tile as
    [V_A | ones | V_B] in bf16.
  phase 2 flash-style attention, no max-subtraction (scores ~ N(0,1)):
    scoresT psum[j, A|B] = kT_j.T @ qT_q (2 heads, K=64 each)
    PT = exp(scores/8): ScalarE reads PSUM, writes bf16 SBUF
    causal: strictly-upper j-blocks skipped, partial-width operands on
      diagonal blocks, one [128,128] triangle mask multiply (GpSimd);
      diagonal blocks processed FIRST so their mask latency hides
      under the long run of full-width blocks.
    acc += [V | ones].T @ PT (bf16, rows 64:128 = softmax denominator)
    The j-loop is software-pipelined: scores(j+1) are emitted before
      attnV(j), so the PE never head-of-line blocks on the exp.
  normalize: 1/l via DVE reciprocal (keeps ScalarE exp-only),
    attnT = acc * (1/l) on DVE.
  phase 3 output projection partial (f32r): partialT[o, s], written
    transposed [1024, 4096] per core; host sums the 8 partials.
"""

import os
import sys

import numpy as np

for _p in ("/opt/trn_rl_repo", "/root/.axon_site/_ro/trn_rl_repo"):
    if os.path.isdir(_p) and _p not in sys.path:
        sys.path.insert(0, _p)

from contextlib import ExitStack

import ml_dtypes

import concourse.bass as bass
import concourse.tile as tile
from concourse import bacc, bass_utils, mybir
from concourse.masks import make_identity
import concourse.hw_specs as _hw_specs
import functools as _functools

# Pin Exp/Ln to the one activation-table set containing both, so the
# softmax exp and the exp(-ln(l)) normalization never ping-pong
# ACT_TABLE_LOADs. Only the *selection* map is filtered; set order
# (= act_func_set_id) is unchanged. (Same mechanism as the original
# baseline kernel.)
_orig_get_tables = _hw_specs.get_activation_tables


@_functools.cache
def _pinned_tables(arch):
    t = dict(_orig_get_tables(arch))
    strip = {mybir.ActivationFunctionType.Exp, mybir.ActivationFunctionType.Ln}
    for name in t:
        if name != "natural_log_exp_and_others":
            t[name] = t[name] - strip
    return t


_hw_specs.get_activation_tables = _pinned_tables
bacc.get_activation_tables = _pinned_tables

# Problem shape (hardcoded per the harness contract).
B, S, D, H = 1, 4096, 1024, 16
HD = D // H          # 64
NCORES = 8
HPC = H // NCORES    # 2 heads per core
M = HPC * HD         # 128 channels per core
SBK = 512            # s/q block size
NSB = S // SBK       # 8
JBK = 128            # j (key) block size
NJT = S // JBK       # 32
VW = 3 * HD          # v_aug row width per j-tile: [V_A | ones | V_B]

F32 = mybir.dt.float32
F32R = mybir.dt.float32r
BF16 = mybir.dt.bfloat16

NP_BF16 = ml_dtypes.bfloat16

_CACHE = {}


def _build_nc():
    """Build + compile the per-core Bass program (identical on all cores)."""
    nc = bacc.Bacc("TRN2", target_bir_lowering=False, debug=False,
                   num_devices=NCORES)

    # bf16 x, pre-transposed: [p, dt, s] with d = dt*128+p
    xb = nc.dram_tensor("xb", [128, 8 * S], BF16, kind="ExternalInput").ap()
    # bf16 weights, [p, dt, m] = W[m_local, dt*128+p]
    wq = nc.dram_tensor("wq", [128, 8 * M], BF16, kind="ExternalInput").ap()
    wk = nc.dram_tensor("wk", [128, 8 * M], BF16, kind="ExternalInput").ap()
    wv = nc.dram_tensor("wv", [128, 8 * M], BF16, kind="ExternalInput").ap()
    wo = nc.dram_tensor("wo", [M, D], F32R, kind="ExternalInput").ap()
    ones = nc.dram_tensor("ones", [128, HD], BF16, kind="ExternalInput").ap()
    dmask = nc.dram_tensor("dmask", [JBK, JBK], BF16,
                           kind="ExternalInput").ap()
    outp = nc.dram_tensor("outp", [D, S], F32, kind="ExternalOutput").ap()

    with tile.TileContext(nc) as tc:
        with ExitStack() as ctx:
            _emit(ctx, tc, nc, xb, wq, wk, wv, wo, ones, dmask, outp)
    nc.compile()
    return nc


def _emit(ctx, tc, nc, xb, wq, wk, wv, wo, ones, dmask, outp):
    const = ctx.enter_context(tc.tile_pool(name="const", bufs=1))
    persist = ctx.enter_context(tc.tile_pool(name="persist", bufs=1))
    xb_pool = ctx.enter_context(tc.tile_pool(name="xb", bufs=3))
    vt_pool = ctx.enter_context(tc.tile_pool(name="vt", bufs=2))
    pt_pool = ctx.enter_context(tc.tile_pool(name="pt", bufs=4))
    ot_pool = ctx.enter_context(tc.tile_pool(name="ot", bufs=8))
    lv_pool = ctx.enter_context(tc.tile_pool(name="lv", bufs=2))
    # PSUM: 16 KB/partition = 8 banks. sc 2x(2 banks) + acc 2x1 + work 2x1.
    ps_sc = ctx.enter_context(tc.tile_pool(name="ps_sc", bufs=2, space="PSUM"))
    ps_acc = ctx.enter_context(tc.tile_pool(name="ps_acc", bufs=2,
                                            space="PSUM"))
    ps_wk = ctx.enter_context(tc.tile_pool(name="ps_wk", bufs=2, space="PSUM"))

    # ---- constants ----
    ident = const.tile([128, 128], F32)
    make_identity(nc, ident)
    wq_sb = const.tile([128, 8 * M], BF16)
    wk_sb = const.tile([128, 8 * M], BF16)
    wv_sb = const.tile([128, 8 * M], BF16)
    wo_sb = const.tile([128, D], F32R)
    mask_sb = const.tile([128, JBK], BF16)
    nc.sync.dma_start(out=wq_sb[:], in_=wq[:])
    nc.sync.dma_start(out=wk_sb[:], in_=wk[:])
    nc.sync.dma_start(out=wv_sb[:], in_=wv[:])
    nc.sync.dma_start(out=mask_sb[:], in_=dmask[:])

    def _late_consts():
        for c in range(4):
            nc.sync.dma_start(out=wo_sb[:, bass.ts(c, 256)],
                              in_=wo[:, bass.ts(c, 256)])

    wq_r = wq_sb[:].rearrange("p (dt m) -> p dt m", dt=8)
    wk_r = wk_sb[:].rearrange("p (dt m) -> p dt m", dt=8)
    wv_r = wv_sb[:].rearrange("p (dt m) -> p dt m", dt=8)

    # ---- persistent SBUF ----
    qT_sb = persist.tile([128, S], BF16)
    kT_sb = persist.tile([128, S], BF16)
    v_aug = persist.tile([128, NJT * VW], BF16)
    attnT = persist.tile([128, S], F32R)

    # per j-tile layout [V_A | ones | V_B]: head A reads cols 0:128
    # ([V_A | 1] -> acc rows 64:128 = denominator l), head B reads cols
    # 64:192 ([1 | V_B] -> acc rows 0:64 = l). ones filled by one
    # broadcast DMA (step-0 middle dim on input).
    v_aug_r = v_aug[:].rearrange("p (t c w) -> p t c w", c=3, w=HD)
    ones_bcast = bass.AP(
        tensor=ones.tensor, offset=0,
        ap=[[HD, 128], [0, NJT], [1, HD]])
    nc.sync.dma_start(out=v_aug_r[:, :, 1, :], in_=ones_bcast)

    xb_r_dram = xb.rearrange("p (dt s) -> p dt s", dt=8)

    def phase1_dma(sb):
        """x loads for s-block sb (issued one attention block early)."""
        xbt = xb_pool.tile([128, 8 * SBK], BF16, tag="xb", name="xbt")
        xb_r = xbt[:].rearrange("p (dt s) -> p dt s", dt=8)
        sl = slice(sb * SBK, (sb + 1) * SBK)
        if sb == 0:
            # dtile-major chunks so the first q matmuls start after ~1
            # chunk instead of the full 8 KB/partition load.
            for d0 in range(0, 8, 2):
                nc.gpsimd.dma_start(out=xb_r[:, d0:d0 + 2, :],
                                    in_=xb_r_dram[:, d0:d0 + 2, sl])
        else:
            nc.gpsimd.dma_start(out=xb_r[:, :4, :], in_=xb_r_dram[:, :4, sl])
            nc.gpsimd.dma_start(out=xb_r[:, 4:, :], in_=xb_r_dram[:, 4:, sl])
        return xb_r

    def phase1_units(sb, xb_r):
        """QKV projections for s-block sb, as PE work units."""
        units = []
        state = {}

        def qk_unit(which, lo):
            def run():
                w_r = {"q": wq_r, "k": wk_r, "v": wv_r}[which]
                if lo == 0:
                    state[which] = ps_wk.tile([128, SBK], F32, tag="work",
                                              name=f"ps_{which}")
                ps = state[which]
                for dt in range(lo, lo + 4):
                    nc.tensor.matmul(ps[:], lhsT=w_r[:, dt, :],
                                     rhs=xb_r[:, dt, :],
                                     start=dt == 0, stop=dt == 7)
                if lo == 4 and which != "v":
                    dst = qT_sb if which == "q" else kT_sb
                    nc.vector.tensor_copy(dst[:, bass.ts(sb, SBK)], ps[:])
            return run

        def v_evict():
            vt = vt_pool.tile([128, SBK], F32, tag="vt", name="vt")
            nc.vector.tensor_copy(vt[:], state["v"][:])
            state["vt"] = vt

        def t_unit():
            tp = ps_wk.tile([128, SBK], F32, tag="work", name="ps_tp")
            for t in range(4):
                jt = sb * 4 + t
                tsl = bass.ts(t, 128)
                nc.tensor.transpose(tp[:, tsl], state["vt"][:, tsl], ident[:])
                tp_r = bass.AP(tensor=tp.tensor, offset=tp.offset + t * 128,
                               ap=[list(tp.ap[0]), [HD, 2], [1, HD]])
                nc.vector.tensor_copy(v_aug_r[:, jt, 0::2, :], tp_r)

        for which in ("q", "k", "v"):
            for lo in (0, 4):
                units.append(qk_unit(which, lo))
        units.append(v_evict)
        units.append(t_unit)
        return units

    def proj_units(qb, tail=False):
        """Output-projection partial for s-block qb -> DRAM (transposed).
        At the tail there is no attention left to hide behind, so borrow
        the (now idle) score-psum banks and the ScalarE for every other
        eviction to shorten the drain."""
        qsl = bass.ts(qb, SBK)
        units = []

        def ob_unit(ob):
            def run():
                if tail and ob % 2 == 0:
                    big = ps_sc.tile([128, 2 * SBK], F32, tag="sc",
                                     name="ps_po_sc")
                    po = big[:, 0:SBK]
                else:
                    po = ps_wk.tile([128, SBK], F32, tag="work",
                                    name="ps_po")[:]
                nc.tensor.matmul(po, lhsT=wo_sb[:, bass.ts(ob, 128)],
                                 rhs=attnT[:, qsl], start=True, stop=True)
                ot = ot_pool.tile([128, SBK], F32, tag="ot")
                if tail and ob % 2 == 1:
                    nc.scalar.activation(ot[:], po,
                                         mybir.ActivationFunctionType.Copy)
                else:
                    nc.vector.tensor_copy(ot[:], po)
                if tail and ob % 3 == 1:
                    nc.gpsimd.dma_start(out=outp[bass.ts(ob, 128), qsl],
                                        in_=ot[:])
                elif tail and ob % 3 == 2:
                    nc.scalar.dma_start(out=outp[bass.ts(ob, 128), qsl],
                                        in_=ot[:])
                else:
                    nc.sync.dma_start(out=outp[bass.ts(ob, 128), qsl],
                                      in_=ot[:])
            return run

        for ob in range(8):
            units.append(ob_unit(ob))
        return units

    scale = float(1.0 / np.sqrt(HD))

    def attention(qb, units):
        """Causal attention for query block qb (both heads), with `units`
        (phase1/proj closures) interleaved into the PE stream."""
        nj = 4 * (qb + 1)               # valid j128-blocks
        # diagonal strips first: their GpSimd mask latency hides under
        # the long tail of full-width blocks.
        order = list(range(nj - 4, nj)) + list(range(0, nj - 4))
        offs = {j: max(0, 128 * (j - (nj - 4))) for j in order}

        acc_A = ps_acc.tile([128, SBK], F32, tag="acc", name="acc_A")
        acc_B = ps_acc.tile([128, SBK], F32, tag="acc", name="acc_B")

        def emit_sc(j):
            off = offs[j]
            sc = ps_sc.tile([128, 2 * SBK], F32, tag="sc", name="sc")
            qa = qT_sb[0:64, qb * SBK + off:(qb + 1) * SBK]
            qb_ap = qT_sb[64:128, qb * SBK + off:(qb + 1) * SBK]
            nc.tensor.matmul(sc[:, off:SBK],
                             lhsT=kT_sb[0:64, bass.ts(j, JBK)],
                             rhs=qa, start=True, stop=True)
            nc.tensor.matmul(sc[:, SBK + off:2 * SBK],
                             lhsT=kT_sb[64:128, bass.ts(j, JBK)],
                             rhs=qb_ap, start=True, stop=True)
            return sc

        def emit_exp(j, sc):
            off = offs[j]
            pt = pt_pool.tile([128, 2 * SBK], BF16, tag="pt", name="pt")
            if off == 0:
                nc.scalar.activation(pt[:], sc[:],
                                     mybir.ActivationFunctionType.Exp,
                                     scale=scale)
            else:
                w = SBK - off
                sc2 = bass.AP(tensor=sc.tensor, offset=sc.offset + off,
                              ap=[list(sc.ap[0]), [SBK, 2], [1, w]])
                pt2 = bass.AP(tensor=pt.tensor, offset=pt.offset + off,
                              ap=[list(pt.ap[0]), [SBK, 2], [1, w]])
                nc.scalar.activation(pt2, sc2,
                                     mybir.ActivationFunctionType.Exp,
                                     scale=scale)
            return pt

        def emit_mask(j, pt):
            r = j - (nj - 4)
            dlo = 128 * r
            nc.gpsimd.tensor_mul(pt[:, dlo:dlo + 128],
                                 pt[:, dlo:dlo + 128], mask_sb[:])
            nc.gpsimd.tensor_mul(pt[:, SBK + dlo:SBK + dlo + 128],
                                 pt[:, SBK + dlo:SBK + dlo + 128],
                                 mask_sb[:])

        def emit_pv(idx, j, pt):
            off = offs[j]
            st, sp = idx == 0, idx == nj - 1
            vb = j * VW
            nc.tensor.matmul(acc_A[:, off:SBK],
                             lhsT=v_aug[:, vb:vb + 128],
                             rhs=pt[:, off:SBK], start=st, stop=sp)
            nc.tensor.matmul(acc_B[:, off:SBK],
                             lhsT=v_aug[:, vb + HD:vb + VW],
                             rhs=pt[:, SBK + off:2 * SBK],
                             start=st, stop=sp)

        # Lookahead-2 software pipeline: pv(j) is emitted one full
        # iteration after exp(j), so its semaphore fired a whole period
        # earlier and the PE queue never head-of-line blocks.  Stalls
        # reset the tensor engine's pstate ramp (1.2 -> 2.4 GHz needs
        # ~3us of continuous execution), so this is what keeps the PE
        # at full clock.
        emitted = 0
        sc = emit_sc(order[0])
        pts = {}
        for idx, j in enumerate(order):
            pts[idx] = emit_exp(j, sc)
            if j >= nj - 4:
                emit_mask(j, pts[idx])
            if idx + 1 < nj:
                sc = emit_sc(order[idx + 1])
            want = ((idx + 1) * len(units)) // (nj + 1)
            while emitted < want:
                units[emitted]()
                emitted += 1
            if idx >= 1:
                emit_pv(idx - 1, order[idx - 1], pts.pop(idx - 1))
        while emitted < len(units):
            units[emitted]()
            emitted += 1
        emit_pv(nj - 1, order[nj - 1], pts.pop(nj - 1))
        return acc_A, acc_B

    def normalize(qb, acc_A, acc_B):
        # head A: out rows 0:64, l rows 64:128; head B flipped.
        # 1/l as exp(-ln(l)) on ScalarE: InstReciprocal on DVE measured
        # 3.4us/op on hardware and reciprocal_approx_fast miscomputes
        # there (CoreSim-only). Exp+Ln share one pinned table set so no
        # ACT_TABLE_LOADs are triggered.
        qsl = bass.ts(qb, SBK)
        lnl_a = lv_pool.tile([64, SBK], F32, tag="lv")
        nc.scalar.activation(lnl_a[:], acc_A[HD:2 * HD, :],
                             mybir.ActivationFunctionType.Ln)
        linv_a = lv_pool.tile([64, SBK], F32, tag="lv2")
        nc.scalar.activation(linv_a[:], lnl_a[:],
                             mybir.ActivationFunctionType.Exp, scale=-1.0)
        nc.vector.tensor_mul(attnT[0:64, qsl], acc_A[0:HD, :], linv_a[:])
        lnl_b = lv_pool.tile([64, SBK], F32, tag="lv")
        nc.scalar.activation(lnl_b[:], acc_B[0:HD, :],
                             mybir.ActivationFunctionType.Ln)
        linv_b = lv_pool.tile([64, SBK], F32, tag="lv2")
        nc.scalar.activation(linv_b[:], lnl_b[:],
                             mybir.ActivationFunctionType.Exp, scale=-1.0)
        nc.vector.tensor_mul(attnT[64:128, qsl], acc_B[HD:2 * HD, :],
                             linv_b[:])

    # prologue: phase1(0) un-interleaved, then the qb loop with
    # phase1(qb+1) + proj(qb-1) threaded into attention(qb)'s j-loop.
    # x loads are prefetched two blocks ahead so phase1 units never wait.
    xb_handles = {0: phase1_dma(0)}
    for u in phase1_units(0, xb_handles[0]):
        u()
    xb_handles[1] = phase1_dma(1)
    _late_consts()
    for qb in range(NSB):
        if qb + 2 < NSB:
            xb_handles[qb + 2] = phase1_dma(qb + 2)
        units = []
        if qb + 1 < NSB:
            units += phase1_units(qb + 1, xb_handles[qb + 1])
        if qb > 0:
            units += proj_units(qb - 1)
        accs = attention(qb, units)
        normalize(qb, *accs)
    for u in proj_units(NSB - 1, tail=True):
        u()


def _host_prep(x, Wq, Wk, Wv, Wo):
    xf = np.ascontiguousarray(x.reshape(S, D)).astype(np.float32)
    xT = xf.T  # [D, S]
    xb = np.ascontiguousarray(
        xT.reshape(8, 128, S).transpose(1, 0, 2)
    ).astype(NP_BF16).reshape(128, 8 * S)

    jj = np.arange(JBK)[:, None]
    qq = np.arange(JBK)[None, :]
    dmask = (jj <= qq).astype(NP_BF16)
    ones_arr = np.ones((128, HD), dtype=NP_BF16)

    def w_prep(W_local):
        # [p, dt, m] = W_local[m, dt*128+p]
        A = W_local.T.reshape(8, 128, M).transpose(1, 0, 2)
        return np.ascontiguousarray(A).astype(NP_BF16).reshape(128, -1)

    in_maps = []
    for c in range(NCORES):
        sl = slice(c * M, (c + 1) * M)
        in_maps.append({
            "xb": xb,
            "wq": w_prep(np.asarray(Wq[sl, :], np.float32)),
            "wk": w_prep(np.asarray(Wk[sl, :], np.float32)),
            "wv": w_prep(np.asarray(Wv[sl, :], np.float32)),
            "wo": np.ascontiguousarray(Wo[:, sl].T).astype(np.float32),
            "ones": ones_arr,
            "dmask": dmask,
        })
    return in_maps


def _run(inputs, trace=False):
    x = np.asarray(inputs["x"], dtype=np.float32)
    Wq = np.asarray(inputs["Wq"], dtype=np.float32)
    Wk = np.asarray(inputs["Wk"], dtype=np.float32)
    Wv = np.asarray(inputs["Wv"], dtype=np.float32)
    Wo = np.asarray(inputs["Wo"], dtype=np.float32)

    if "nc" not in _CACHE:
        _CACHE["nc"] = _build_nc()
    nc = _CACHE["nc"]

    in_maps = _host_prep(x, Wq, Wk, Wv, Wo)
    res = bass_utils.run_bass_kernel_spmd(
        nc, in_maps, core_ids=list(range(NCORES)), trace=trace)
    partial = np.zeros((D, S), dtype=np.float64)
    for c in range(NCORES):
        partial += res.results[c]["outp"].astype(np.float64)
    out = partial.T.astype(np.float32).reshape(B, S, D)
    return out, res


def kernel(x, mask, Wq, Wk, Wv, Wo):
    mask = np.asarray(mask)
    causal = np.tril(np.ones((S, S), dtype=bool))
    if mask.reshape(S, S).shape == causal.shape and bool(
            np.array_equal(mask.reshape(S, S), causal)):
        out, _ = _run({"x": x, "Wq": Wq, "Wk": Wk, "Wv": Wv, "Wo": Wo})
        return out
    # safety net for a non-causal mask: exact numpy fallback
    return _numpy_ref(np.asarray(x, np.float32), mask,
                      np.asarray(Wq, np.float32), np.asarray(Wk, np.float32),
                      np.asarray(Wv, np.float32), np.asarray(Wo, np.float32))


def _numpy_ref(x, mask, Wq, Wk, Wv, Wo):
    xf = x.reshape(S, D)
    q = xf @ Wq.T
    k = xf @ Wk.T
    v = xf @ Wv.T
    m2 = mask.reshape(S, S)
    o = np.empty((S, D), dtype=np.float32)
    for h in range(H):
        hs = slice(h * HD, (h + 1) * HD)
        sc = (q[:, hs] @ k[:, hs].T) / np.sqrt(np.float32(HD))
        sc = np.where(m2, sc, np.float32(-1e9))
        sc -= sc.max(axis=-1, keepdims=True)
        p = np.exp(sc)
        p /= p.sum(axis=-1, keepdims=True)
        o[:, hs] = p @ v[:, hs]
    return (o @ Wo.T).astype(np.float32).reshape(B, S, D)


# revision 34
# speedup vs baseline: 1.3326x; 1.0019x over previous
"""Multi-head causal attention (B=1, S=4096, D=1024, H=16, HD=64) on 8
Trainium2 NeuronCores.

Sharding: head-parallel - 16 heads / 8 cores = 2 heads per core (one
128-channel slice of the QKV/output projections per core).

Per-core pipeline (contraction dims on SBUF partitions; softmax exp
reads PSUM directly; all attention matmuls in bf16 - fp8 was measured
to put ~5% relative error on the output because softmax-weight noise
is not damped by the value average):

  phase 1 (interleaved into the attention j-loop as filler PE units):
    qT/kT [128, S] bf16 via wq-stationary matmuls (outputs transposed),
    V via vT matmuls + PE transpose per 128-tile, stored per j-tile as
    [V_A | ones | V_B] in bf16.
  phase 2 flash-style attention, no max-subtraction (scores ~ N(0,1)):
    scoresT psum[j, A|B] = kT_j.T @ qT_q (2 heads, K=64 each)
    PT = exp(scores/8): ScalarE reads PSUM, writes bf16 SBUF
    causal: strictly-upper j-blocks skipped, partial-width operands on
      diagonal blocks, one [128,128] triangle mask multiply (GpSimd);
      diagonal blocks processed FIRST so their mask latency hides
      under the long run of full-width blocks.
    acc += [V | ones].T @ PT (bf16, rows 64:128 = softmax denominator)
    The j-loop is software-pipelined: scores(j+1) are emitted before
      attnV(j), so the PE never head-of-line blocks on the exp.
  normalize: 1/l via DVE reciprocal (keeps ScalarE exp-only),
    attnT = acc * (1/l) on DVE.
  phase 3 output projection partial (f32r): partialT[o, s], written
    transposed [1024, 4096] per core; host sums the 8 partials.
"""

import os
import sys

import numpy as np

for _p in ("/opt/trn_rl_repo", "/root/.axon_site/_ro/trn_rl_repo"):
    if os.path.isdir(_p) and _p not in sys.path:
        sys.path.insert(0, _p)

from contextlib import ExitStack

import ml_dtypes

import concourse.bass as bass
import concourse.tile as tile
from concourse import bacc, bass_utils, mybir
from concourse.masks import make_identity
import concourse.hw_specs as _hw_specs
import functools as _functools

# Pin Exp/Ln to the one activation-table set containing both, so the
# softmax exp and the exp(-ln(l)) normalization never ping-pong
# ACT_TABLE_LOADs. Only the *selection* map is filtered; set order
# (= act_func_set_id) is unchanged. (Same mechanism as the original
# baseline kernel.)
_orig_get_tables = _hw_specs.get_activation_tables


@_functools.cache
def _pinned_tables(arch):
    t = dict(_orig_get_tables(arch))
    strip = {mybir.ActivationFunctionType.Exp, mybir.ActivationFunctionType.Ln}
    for name in t:
        if name != "natural_log_exp_and_others":
            t[name] = t[name] - strip
    return t


_hw_specs.get_activation_tables = _pinned_tables
bacc.get_activation_tables = _pinned_tables

# Problem shape (hardcoded per the harness contract).
B, S, D, H = 1, 4096, 1024, 16
HD = D // H          # 64
NCORES = 8
HPC = H // NCORES    # 2 heads per core
M = HPC * HD         # 128 channels per core
SBK = 512            # s/q block size
NSB = S // SBK       # 8
JBK = 128            # j (key) block size
NJT = S // JBK       # 32
VW = 3 * HD          # v_aug row width per j-tile: [V_A | ones | V_B]

F32 = mybir.dt.float32
F32R = mybir.dt.float32r
BF16 = mybir.dt.bfloat16

NP_BF16 = ml_dtypes.bfloat16

_CACHE = {}


def _build_nc():
    """Build + compile the per-core Bass program (identical on all cores)."""
    nc = bacc.Bacc("TRN2", target_bir_lowering=False, debug=False,
                   num_devices=NCORES)

    # bf16 x, pre-transposed: [p, dt, s] with d = dt*128+p
    xb = nc.dram_tensor("xb", [128, 8 * S], BF16, kind="ExternalInput").ap()
    # bf16 weights, [p, dt, m] = W[m_local, dt*128+p]
    wq = nc.dram_tensor("wq", [128, 8 * M], BF16, kind="ExternalInput").ap()
    wk = nc.dram_tensor("wk", [128, 8 * M], BF16, kind="ExternalInput").ap()
    wv = nc.dram_tensor("wv", [128, 8 * M], BF16, kind="ExternalInput").ap()
    wo = nc.dram_tensor("wo", [M, D], F32R, kind="ExternalInput").ap()
    ones = nc.dram_tensor("ones", [128, HD], BF16, kind="ExternalInput").ap()
    dmask = nc.dram_tensor("dmask", [JBK, JBK], BF16,
                           kind="ExternalInput").ap()
    outp = nc.dram_tensor("outp", [D, S], F32, kind="ExternalOutput").ap()

    with tile.TileContext(nc) as tc:
        with ExitStack() as ctx:
            _emit(ctx, tc, nc, xb, wq, wk, wv, wo, ones, dmask, outp)
    nc.compile()
    return nc


def _emit(ctx, tc, nc, xb, wq, wk, wv, wo, ones, dmask, outp):
    const = ctx.enter_context(tc.tile_pool(name="const", bufs=1))
    persist = ctx.enter_context(tc.tile_pool(name="persist", bufs=1))
    xb_pool = ctx.enter_context(tc.tile_pool(name="xb", bufs=3))
    vt_pool = ctx.enter_context(tc.tile_pool(name="vt", bufs=2))
    pt_pool = ctx.enter_context(tc.tile_pool(name="pt", bufs=4))
    ot_pool = ctx.enter_context(tc.tile_pool(name="ot", bufs=8))
    lv_pool = ctx.enter_context(tc.tile_pool(name="lv", bufs=2))
    # PSUM: 16 KB/partition = 8 banks. sc 2x(2 banks) + acc 2x1 + work 2x1.
    ps_sc = ctx.enter_context(tc.tile_pool(name="ps_sc", bufs=2, space="PSUM"))
    ps_acc = ctx.enter_context(tc.tile_pool(name="ps_acc", bufs=2,
                                            space="PSUM"))
    ps_wk = ctx.enter_context(tc.tile_pool(name="ps_wk", bufs=2, space="PSUM"))

    # ---- constants ----
    ident = const.tile([128, 128], F32)
    make_identity(nc, ident)
    # warm the ScalarE Exp table at t=0 so the first real softmax exp
    # does not pay the 1.3us ACT_TABLE_LOAD on the critical path.
    warm = const.tile([64, 4], F32)
    nc.scalar.activation(warm[:], ident[0:64, 0:4],
                         mybir.ActivationFunctionType.Exp)
    wq_sb = const.tile([128, 8 * M], BF16)
    wk_sb = const.tile([128, 8 * M], BF16)
    wv_sb = const.tile([128, 8 * M], BF16)
    wo_sb = const.tile([128, D], F32R)
    mask_sb = const.tile([128, JBK], BF16)
    nc.sync.dma_start(out=wq_sb[:], in_=wq[:])
    nc.sync.dma_start(out=wk_sb[:], in_=wk[:])
    nc.sync.dma_start(out=wv_sb[:], in_=wv[:])
    nc.sync.dma_start(out=mask_sb[:], in_=dmask[:])

    def _late_consts():
        for c in range(4):
            nc.sync.dma_start(out=wo_sb[:, bass.ts(c, 256)],
                              in_=wo[:, bass.ts(c, 256)])

    wq_r = wq_sb[:].rearrange("p (dt m) -> p dt m", dt=8)
    wk_r = wk_sb[:].rearrange("p (dt m) -> p dt m", dt=8)
    wv_r = wv_sb[:].rearrange("p (dt m) -> p dt m", dt=8)

    # ---- persistent SBUF ----
    qT_sb = persist.tile([128, S], BF16)
    kT_sb = persist.tile([128, S], BF16)
    v_aug = persist.tile([128, NJT * VW], BF16)
    attnT = persist.tile([128, S], F32R)

    # per j-tile layout [V_A | ones | V_B]: head A reads cols 0:128
    # ([V_A | 1] -> acc rows 64:128 = denominator l), head B reads cols
    # 64:192 ([1 | V_B] -> acc rows 0:64 = l). ones filled by one
    # broadcast DMA (step-0 middle dim on input).
    v_aug_r = v_aug[:].rearrange("p (t c w) -> p t c w", c=3, w=HD)
    ones_bcast = bass.AP(
        tensor=ones.tensor, offset=0,
        ap=[[HD, 128], [0, NJT], [1, HD]])
    nc.sync.dma_start(out=v_aug_r[:, :, 1, :], in_=ones_bcast)

    xb_r_dram = xb.rearrange("p (dt s) -> p dt s", dt=8)

    def phase1_dma(sb):
        """x loads for s-block sb (issued one attention block early)."""
        xbt = xb_pool.tile([128, 8 * SBK], BF16, tag="xb", name="xbt")
        xb_r = xbt[:].rearrange("p (dt s) -> p dt s", dt=8)
        sl = slice(sb * SBK, (sb + 1) * SBK)
        # alternate DMA queues per s-block so consecutive prefetches do
        # not serialize behind each other on one DGE.
        eng = nc.gpsimd if sb % 2 == 0 else nc.sync
        if sb == 0:
            # dtile-major chunks so the first q matmuls start after ~1
            # chunk instead of the full 8 KB/partition load.
            for d0 in range(0, 8, 2):
                eng.dma_start(out=xb_r[:, d0:d0 + 2, :],
                              in_=xb_r_dram[:, d0:d0 + 2, sl])
        else:
            eng.dma_start(out=xb_r[:, :4, :], in_=xb_r_dram[:, :4, sl])
            eng.dma_start(out=xb_r[:, 4:, :], in_=xb_r_dram[:, 4:, sl])
        return xb_r

    def phase1_units(sb, xb_r):
        """QKV projections for s-block sb, as PE work units."""
        units = []
        state = {}

        def qk_unit(which, lo):
            def run():
                w_r = {"q": wq_r, "k": wk_r, "v": wv_r}[which]
                if lo == 0:
                    state[which] = ps_wk.tile([128, SBK], F32, tag="work",
                                              name=f"ps_{which}")
                ps = state[which]
                for dt in range(lo, lo + 4):
                    nc.tensor.matmul(ps[:], lhsT=w_r[:, dt, :],
                                     rhs=xb_r[:, dt, :],
                                     start=dt == 0, stop=dt == 7)
                if lo == 4 and which != "v":
                    dst = qT_sb if which == "q" else kT_sb
                    nc.vector.tensor_copy(dst[:, bass.ts(sb, SBK)], ps[:])
            return run

        def v_evict():
            vt = vt_pool.tile([128, SBK], F32, tag="vt", name="vt")
            nc.vector.tensor_copy(vt[:], state["v"][:])
            state["vt"] = vt

        def t_unit():
            tp = ps_wk.tile([128, SBK], F32, tag="work", name="ps_tp")
            for t in range(4):
                jt = sb * 4 + t
                tsl = bass.ts(t, 128)
                nc.tensor.transpose(tp[:, tsl], state["vt"][:, tsl], ident[:])
                tp_r = bass.AP(tensor=tp.tensor, offset=tp.offset + t * 128,
                               ap=[list(tp.ap[0]), [HD, 2], [1, HD]])
                nc.vector.tensor_copy(v_aug_r[:, jt, 0::2, :], tp_r)

        for which in ("q", "k", "v"):
            for lo in (0, 4):
                units.append(qk_unit(which, lo))
        units.append(v_evict)
        units.append(t_unit)
        return units

    def proj_units(qb, tail=False):
        """Output-projection partial for s-block qb -> DRAM (transposed).
        At the tail there is no attention left to hide behind, so borrow
        the (now idle) score-psum banks and the ScalarE for every other
        eviction to shorten the drain."""
        qsl = bass.ts(qb, SBK)
        units = []

        def ob_unit(ob):
            def run():
                if tail and ob % 2 == 0:
                    big = ps_sc.tile([128, 2 * SBK], F32, tag="sc",
                                     name="ps_po_sc")
                    po = big[:, 0:SBK]
                else:
                    po = ps_wk.tile([128, SBK], F32, tag="work",
                                    name="ps_po")[:]
                nc.tensor.matmul(po, lhsT=wo_sb[:, bass.ts(ob, 128)],
                                 rhs=attnT[:, qsl], start=True, stop=True)
                ot = ot_pool.tile([128, SBK], F32, tag="ot")
                if tail and ob % 2 == 1:
                    nc.scalar.activation(ot[:], po,
                                         mybir.ActivationFunctionType.Copy)
                else:
                    nc.vector.tensor_copy(ot[:], po)
                if tail and ob % 3 == 1:
                    nc.gpsimd.dma_start(out=outp[bass.ts(ob, 128), qsl],
                                        in_=ot[:])
                elif tail and ob % 3 == 2:
                    nc.scalar.dma_start(out=outp[bass.ts(ob, 128), qsl],
                                        in_=ot[:])
                else:
                    nc.sync.dma_start(out=outp[bass.ts(ob, 128), qsl],
                                      in_=ot[:])
            return run

        for ob in range(8):
            units.append(ob_unit(ob))
        return units

    scale = float(1.0 / np.sqrt(HD))

    def attention(qb, units):
        """Causal attention for query block qb (both heads), with `units`
        (phase1/proj closures) interleaved into the PE stream."""
        nj = 4 * (qb + 1)               # valid j128-blocks
        # diagonal strips first: their GpSimd mask latency hides under
        # the long tail of full-width blocks.
        order = list(range(nj - 4, nj)) + list(range(0, nj - 4))
        offs = {j: max(0, 128 * (j - (nj - 4))) for j in order}

        acc_A = ps_acc.tile([128, SBK], F32, tag="acc", name="acc_A")
        acc_B = ps_acc.tile([128, SBK], F32, tag="acc", name="acc_B")

        def emit_sc(j):
            off = offs[j]
            sc = ps_sc.tile([128, 2 * SBK], F32, tag="sc", name="sc")
            qa = qT_sb[0:64, qb * SBK + off:(qb + 1) * SBK]
            qb_ap = qT_sb[64:128, qb * SBK + off:(qb + 1) * SBK]
            nc.tensor.matmul(sc[:, off:SBK],
                             lhsT=kT_sb[0:64, bass.ts(j, JBK)],
                             rhs=qa, start=True, stop=True)
            nc.tensor.matmul(sc[:, SBK + off:2 * SBK],
                             lhsT=kT_sb[64:128, bass.ts(j, JBK)],
                             rhs=qb_ap, start=True, stop=True)
            return sc

        def emit_exp(j, sc):
            off = offs[j]
            pt = pt_pool.tile([128, 2 * SBK], BF16, tag="pt", name="pt")
            if off == 0:
                nc.scalar.activation(pt[:], sc[:],
                                     mybir.ActivationFunctionType.Exp,
                                     scale=scale)
            else:
                w = SBK - off
                sc2 = bass.AP(tensor=sc.tensor, offset=sc.offset + off,
                              ap=[list(sc.ap[0]), [SBK, 2], [1, w]])
                pt2 = bass.AP(tensor=pt.tensor, offset=pt.offset + off,
                              ap=[list(pt.ap[0]), [SBK, 2], [1, w]])
                nc.scalar.activation(pt2, sc2,
                                     mybir.ActivationFunctionType.Exp,
                                     scale=scale)
            return pt

        def emit_mask(j, pt):
            # bf16 SBUF operands hit the DVE 2x mode (~190ns); also keeps
            # the Pool engine free for its software-DGE duties.
            r = j - (nj - 4)
            dlo = 128 * r
            nc.vector.tensor_mul(pt[:, dlo:dlo + 128],
                                 pt[:, dlo:dlo + 128], mask_sb[:])
            nc.vector.tensor_mul(pt[:, SBK + dlo:SBK + dlo + 128],
                                 pt[:, SBK + dlo:SBK + dlo + 128],
                                 mask_sb[:])

        def emit_pv(idx, j, pt):
            off = offs[j]
            st, sp = idx == 0, idx == nj - 1
            vb = j * VW
            nc.tensor.matmul(acc_A[:, off:SBK],
                             lhsT=v_aug[:, vb:vb + 128],
                             rhs=pt[:, off:SBK], start=st, stop=sp)
            nc.tensor.matmul(acc_B[:, off:SBK],
                             lhsT=v_aug[:, vb + HD:vb + VW],
                             rhs=pt[:, SBK + off:2 * SBK],
                             start=st, stop=sp)

        # Lookahead-2 software pipeline: pv(j) is emitted one full
        # iteration after exp(j), so its semaphore fired a whole period
        # earlier and the PE queue never head-of-line blocks.  Stalls
        # reset the tensor engine's pstate ramp (1.2 -> 2.4 GHz needs
        # ~3us of continuous execution), so this is what keeps the PE
        # at full clock.
        emitted = 0
        sc = emit_sc(order[0])
        pts = {}
        for idx, j in enumerate(order):
            pts[idx] = emit_exp(j, sc)
            if j >= nj - 4:
                emit_mask(j, pts[idx])
            if idx + 1 < nj:
                sc = emit_sc(order[idx + 1])
            want = ((idx + 1) * len(units)) // (nj + 1)
            while emitted < want:
                units[emitted]()
                emitted += 1
            if idx >= 1:
                emit_pv(idx - 1, order[idx - 1], pts.pop(idx - 1))
        while emitted < len(units):
            units[emitted]()
            emitted += 1
        emit_pv(nj - 1, order[nj - 1], pts.pop(nj - 1))
        return acc_A, acc_B

    def normalize(qb, acc_A, acc_B):
        # head A: out rows 0:64, l rows 64:128; head B flipped.
        # 1/l as exp(-ln(l)) on ScalarE (Exp+Ln share one pinned table
        # set so no ACT_TABLE_LOADs). DVE InstReciprocal costs 3.4us/op
        # on hardware; DVE divide and reciprocal_approx_fast fail on
        # hardware (sim-only).
        qsl = bass.ts(qb, SBK)
        lnl_a = lv_pool.tile([64, SBK], F32, tag="lv")
        nc.scalar.activation(lnl_a[:], acc_A[HD:2 * HD, :],
                             mybir.ActivationFunctionType.Ln)
        linv_a = lv_pool.tile([64, SBK], F32, tag="lv2")
        nc.scalar.activation(linv_a[:], lnl_a[:],
                             mybir.ActivationFunctionType.Exp, scale=-1.0)
        nc.vector.tensor_mul(attnT[0:64, qsl], acc_A[0:HD, :], linv_a[:])
        lnl_b = lv_pool.tile([64, SBK], F32, tag="lv")
        nc.scalar.activation(lnl_b[:], acc_B[0:HD, :],
                             mybir.ActivationFunctionType.Ln)
        linv_b = lv_pool.tile([64, SBK], F32, tag="lv2")
        nc.scalar.activation(linv_b[:], lnl_b[:],
                             mybir.ActivationFunctionType.Exp, scale=-1.0)
        nc.vector.tensor_mul(attnT[64:128, qsl], acc_B[HD:2 * HD, :],
                             linv_b[:])

    # prologue: phase1(0) un-interleaved, then the qb loop with
    # phase1(qb+1) + proj(qb-1) threaded into attention(qb)'s j-loop.
    # x loads are prefetched two blocks ahead so phase1 units never wait.
    xb_handles = {0: phase1_dma(0)}
    for u in phase1_units(0, xb_handles[0]):
        u()
    xb_handles[1] = phase1_dma(1)
    _late_consts()
    for qb in range(NSB):
        if qb + 2 < NSB:
            xb_handles[qb + 2] = phase1_dma(qb + 2)
        units = []
        if qb + 1 < NSB:
            units += phase1_units(qb + 1, xb_handles[qb + 1])
        if qb > 0:
            units += proj_units(qb - 1)
        accs = attention(qb, units)
        normalize(qb, *accs)
    for u in proj_units(NSB - 1, tail=True):
        u()


def _host_prep(x, Wq, Wk, Wv, Wo):
    xf = np.ascontiguousarray(x.reshape(S, D)).astype(np.float32)
    xT = xf.T  # [D, S]
    xb = np.ascontiguousarray(
        xT.reshape(8, 128, S).transpose(1, 0, 2)
    ).astype(NP_BF16).reshape(128, 8 * S)

    jj = np.arange(JBK)[:, None]
    qq = np.arange(JBK)[None, :]
    dmask = (jj <= qq).astype(NP_BF16)
    ones_arr = np.ones((128, HD), dtype=NP_BF16)

    def w_prep(W_local):
        # [p, dt, m] = W_local[m, dt*128+p]
        A = W_local.T.reshape(8, 128, M).transpose(1, 0, 2)
        return np.ascontiguousarray(A).astype(NP_BF16).reshape(128, -1)

    in_maps = []
    for c in range(NCORES):
        sl = slice(c * M, (c + 1) * M)
        in_maps.append({
            "xb": xb,
            "wq": w_prep(np.asarray(Wq[sl, :], np.float32)),
            "wk": w_prep(np.asarray(Wk[sl, :], np.float32)),
            "wv": w_prep(np.asarray(Wv[sl, :], np.float32)),
            "wo": np.ascontiguousarray(Wo[:, sl].T).astype(np.float32),
            "ones": ones_arr,
            "dmask": dmask,
        })
    return in_maps


def _run(inputs, trace=False):
    x = np.asarray(inputs["x"], dtype=np.float32)
    Wq = np.asarray(inputs["Wq"], dtype=np.float32)
    Wk = np.asarray(inputs["Wk"], dtype=np.float32)
    Wv = np.asarray(inputs["Wv"], dtype=np.float32)
    Wo = np.asarray(inputs["Wo"], dtype=np.float32)

    if "nc" not in _CACHE:
        _CACHE["nc"] = _build_nc()
    nc = _CACHE["nc"]

    in_maps = _host_prep(x, Wq, Wk, Wv, Wo)
    res = bass_utils.run_bass_kernel_spmd(
        nc, in_maps, core_ids=list(range(NCORES)), trace=trace)
    partial = np.zeros((D, S), dtype=np.float64)
    for c in range(NCORES):
        partial += res.results[c]["outp"].astype(np.float64)
    out = partial.T.astype(np.float32).reshape(B, S, D)
    return out, res


def kernel(x, mask, Wq, Wk, Wv, Wo):
    mask = np.asarray(mask)
    causal = np.tril(np.ones((S, S), dtype=bool))
    if mask.reshape(S, S).shape == causal.shape and bool(
            np.array_equal(mask.reshape(S, S), causal)):
        out, _ = _run({"x": x, "Wq": Wq, "Wk": Wk, "Wv": Wv, "Wo": Wo})
        return out
    # safety net for a non-causal mask: exact numpy fallback
    return _numpy_ref(np.asarray(x, np.float32), mask,
                      np.asarray(Wq, np.float32), np.asarray(Wk, np.float32),
                      np.asarray(Wv, np.float32), np.asarray(Wo, np.float32))


def _numpy_ref(x, mask, Wq, Wk, Wv, Wo):
    xf = x.reshape(S, D)
    q = xf @ Wq.T
    k = xf @ Wk.T
    v = xf @ Wv.T
    m2 = mask.reshape(S, S)
    o = np.empty((S, D), dtype=np.float32)
    for h in range(H):
        hs = slice(h * HD, (h + 1) * HD)
        sc = (q[:, hs] @ k[:, hs].T) / np.sqrt(np.float32(HD))
        sc = np.where(m2, sc, np.float32(-1e9))
        sc -= sc.max(axis=-1, keepdims=True)
        p = np.exp(sc)
        p /= p.sum(axis=-1, keepdims=True)
        o[:, hs] = p @ v[:, hs]
    return (o @ Wo.T).astype(np.float32).reshape(B, S, D)


# revision 37
# speedup vs baseline: 1.3508x; 1.0137x over previous
"""Multi-head causal attention (B=1, S=4096, D=1024, H=16, HD=64) on 8
Trainium2 NeuronCores.

Sharding: head-parallel - 16 heads / 8 cores = 2 heads per core (one
128-channel slice of the QKV/output projections per core).

Per-core pipeline (contraction dims on SBUF partitions; softmax exp
reads PSUM directly; all attention matmuls in bf16 - fp8 was measured
to put ~5% relative error on the output because softmax-weight noise
is not damped by the value average):

  phase 1 (interleaved into the attention j-loop as filler PE units):
    qT/kT [128, S] bf16 via wq-stationary matmuls (outputs transposed),
    V via vT matmuls + PE transpose per 128-tile, stored per j-tile as
    [V_A | ones | V_B] in bf16.
  phase 2 flash-style attention, no max-subtraction (scores ~ N(0,1)):
    scoresT psum[j, A|B] = kT_j.T @ qT_q (2 heads, K=64 each)
    PT = exp(scores/8): ScalarE reads PSUM, writes bf16 SBUF
    causal: strictly-upper j-blocks skipped, partial-width operands on
      diagonal blocks, one [128,128] triangle mask multiply (GpSimd);
      diagonal blocks processed FIRST so their mask latency hides
      under the long run of full-width blocks.
    acc += [V | ones].T @ PT (bf16, rows 64:128 = softmax denominator)
    The j-loop is software-pipelined: scores(j+1) are emitted before
      attnV(j), so the PE never head-of-line blocks on the exp.
  normalize: 1/l via DVE reciprocal (keeps ScalarE exp-only),
    attnT = acc * (1/l) on DVE.
  phase 3 output projection partial (f32r): partialT[o, s], written
    transposed [1024, 4096] per core; host sums the 8 partials.
"""

import os
import sys

import numpy as np

for _p in ("/opt/trn_rl_repo", "/root/.axon_site/_ro/trn_rl_repo"):
    if os.path.isdir(_p) and _p not in sys.path:
        sys.path.insert(0, _p)

from contextlib import ExitStack

import ml_dtypes

import concourse.bass as bass
import concourse.tile as tile
from concourse import bacc, bass_utils, mybir
from concourse.masks import make_identity
import concourse.hw_specs as _hw_specs
import functools as _functools

# Pin Exp/Ln to the one activation-table set containing both, so the
# softmax exp and the exp(-ln(l)) normalization never ping-pong
# ACT_TABLE_LOADs. Only the *selection* map is filtered; set order
# (= act_func_set_id) is unchanged. (Same mechanism as the original
# baseline kernel.)
_orig_get_tables = _hw_specs.get_activation_tables


@_functools.cache
def _pinned_tables(arch):
    t = dict(_orig_get_tables(arch))
    strip = {mybir.ActivationFunctionType.Exp, mybir.ActivationFunctionType.Ln}
    for name in t:
        if name != "natural_log_exp_and_others":
            t[name] = t[name] - strip
    return t


_hw_specs.get_activation_tables = _pinned_tables
bacc.get_activation_tables = _pinned_tables

# Problem shape (hardcoded per the harness contract).
B, S, D, H = 1, 4096, 1024, 16
HD = D // H          # 64
NCORES = 8
HPC = H // NCORES    # 2 heads per core
M = HPC * HD         # 128 channels per core
SBK = 512            # s/q block size
NSB = S // SBK       # 8
JBK = 128            # j (key) block size
NJT = S // JBK       # 32
VW = 3 * HD          # v_aug row width per j-tile: [V_A | ones | V_B]

F32 = mybir.dt.float32
F32R = mybir.dt.float32r
BF16 = mybir.dt.bfloat16

NP_BF16 = ml_dtypes.bfloat16

_CACHE = {}


def _build_nc():
    """Build + compile the per-core Bass program (identical on all cores)."""
    nc = bacc.Bacc("TRN2", target_bir_lowering=False, debug=False,
                   num_devices=NCORES)

    # bf16 x, pre-transposed: [p, dt, s] with d = dt*128+p
    xb = nc.dram_tensor("xb", [128, 8 * S], BF16, kind="ExternalInput").ap()
    # bf16 weights, [p, dt, m] = W[m_local, dt*128+p]
    wq = nc.dram_tensor("wq", [128, 8 * M], BF16, kind="ExternalInput").ap()
    wk = nc.dram_tensor("wk", [128, 8 * M], BF16, kind="ExternalInput").ap()
    wv = nc.dram_tensor("wv", [128, 8 * M], BF16, kind="ExternalInput").ap()
    wo = nc.dram_tensor("wo", [M, D], F32R, kind="ExternalInput").ap()
    ones = nc.dram_tensor("ones", [128, HD], BF16, kind="ExternalInput").ap()
    dmask = nc.dram_tensor("dmask", [JBK, JBK], BF16,
                           kind="ExternalInput").ap()
    outp = nc.dram_tensor("outp", [D, S], F32, kind="ExternalOutput").ap()

    with tile.TileContext(nc) as tc:
        with ExitStack() as ctx:
            _emit(ctx, tc, nc, xb, wq, wk, wv, wo, ones, dmask, outp)
    nc.compile()
    return nc


def _emit(ctx, tc, nc, xb, wq, wk, wv, wo, ones, dmask, outp):
    const = ctx.enter_context(tc.tile_pool(name="const", bufs=1))
    persist = ctx.enter_context(tc.tile_pool(name="persist", bufs=1))
    xb_pool = ctx.enter_context(tc.tile_pool(name="xb", bufs=3))
    vt_pool = ctx.enter_context(tc.tile_pool(name="vt", bufs=2))
    pt_pool = ctx.enter_context(tc.tile_pool(name="pt", bufs=4))
    ot_pool = ctx.enter_context(tc.tile_pool(name="ot", bufs=8))
    lv_pool = ctx.enter_context(tc.tile_pool(name="lv", bufs=2))
    # PSUM: 16 KB/partition = 8 banks. sc 2x(2 banks) + acc 2x1 + work 2x1.
    ps_sc = ctx.enter_context(tc.tile_pool(name="ps_sc", bufs=2, space="PSUM"))
    ps_acc = ctx.enter_context(tc.tile_pool(name="ps_acc", bufs=2,
                                            space="PSUM"))
    ps_wk = ctx.enter_context(tc.tile_pool(name="ps_wk", bufs=2, space="PSUM"))

    # ---- constants ----
    ident = const.tile([128, 128], F32)
    make_identity(nc, ident)
    # warm the ScalarE Exp table at t=0 so the first real softmax exp
    # does not pay the 1.3us ACT_TABLE_LOAD on the critical path.
    warm = const.tile([64, 4], F32)
    nc.scalar.activation(warm[:], ident[0:64, 0:4],
                         mybir.ActivationFunctionType.Exp)
    wq_sb = const.tile([128, 8 * M], BF16)
    wk_sb = const.tile([128, 8 * M], BF16)
    wv_sb = const.tile([128, 8 * M], BF16)
    wo_sb = const.tile([128, D], F32R)
    mask_sb = const.tile([128, JBK], BF16)
    # startup-critical sync-queue order: wq, wk (phase1(0) q/k), then
    # the first half of xb(1) slots in before wv; mask/ones before the
    # first exp/pv. xb(0) flows on the gpsimd queue in parallel.
    nc.sync.dma_start(out=wq_sb[:], in_=wq[:])
    nc.sync.dma_start(out=wk_sb[:], in_=wk[:])

    def _late_consts():
        for c in range(4):
            nc.sync.dma_start(out=wo_sb[:, bass.ts(c, 256)],
                              in_=wo[:, bass.ts(c, 256)])

    wq_r = wq_sb[:].rearrange("p (dt m) -> p dt m", dt=8)
    wk_r = wk_sb[:].rearrange("p (dt m) -> p dt m", dt=8)
    wv_r = wv_sb[:].rearrange("p (dt m) -> p dt m", dt=8)

    # ---- persistent SBUF ----
    qT_sb = persist.tile([128, S], BF16)
    kT_sb = persist.tile([128, S], BF16)
    v_aug = persist.tile([128, NJT * VW], BF16)
    attnT = persist.tile([128, S], F32R)

    # per j-tile layout [V_A | ones | V_B]: head A reads cols 0:128
    # ([V_A | 1] -> acc rows 64:128 = denominator l), head B reads cols
    # 64:192 ([1 | V_B] -> acc rows 0:64 = l). ones filled by one
    # broadcast DMA (step-0 middle dim on input).
    v_aug_r = v_aug[:].rearrange("p (t c w) -> p t c w", c=3, w=HD)
    ones_bcast = bass.AP(
        tensor=ones.tensor, offset=0,
        ap=[[HD, 128], [0, NJT], [1, HD]])

    xb_r_dram = xb.rearrange("p (dt s) -> p dt s", dt=8)

    def phase1_dma(sb):
        """x loads for s-block sb (issued one attention block early)."""
        xbt = xb_pool.tile([128, 8 * SBK], BF16, tag="xb", name="xbt")
        xb_r = xbt[:].rearrange("p (dt s) -> p dt s", dt=8)
        sl = slice(sb * SBK, (sb + 1) * SBK)
        # alternate DMA queues per s-block so consecutive prefetches do
        # not serialize behind each other on one DGE.
        eng = nc.gpsimd if sb % 2 == 0 else nc.sync
        if sb == 0:
            # dtile-major chunks so the first q matmuls start after ~1
            # chunk instead of the full 8 KB/partition load.
            for d0 in range(0, 8, 2):
                eng.dma_start(out=xb_r[:, d0:d0 + 2, :],
                              in_=xb_r_dram[:, d0:d0 + 2, sl])
        else:
            eng.dma_start(out=xb_r[:, :4, :], in_=xb_r_dram[:, :4, sl])
            eng.dma_start(out=xb_r[:, 4:, :], in_=xb_r_dram[:, 4:, sl])
        return xb_r

    def phase1_units(sb, xb_r):
        """QKV projections for s-block sb, as PE work units."""
        units = []
        state = {}

        def qk_unit(which, lo):
            def run():
                w_r = {"q": wq_r, "k": wk_r, "v": wv_r}[which]
                if lo == 0:
                    state[which] = ps_wk.tile([128, SBK], F32, tag="work",
                                              name=f"ps_{which}")
                ps = state[which]
                for dt in range(lo, lo + 4):
                    nc.tensor.matmul(ps[:], lhsT=w_r[:, dt, :],
                                     rhs=xb_r[:, dt, :],
                                     start=dt == 0, stop=dt == 7)
                if lo == 4 and which != "v":
                    dst = qT_sb if which == "q" else kT_sb
                    nc.vector.tensor_copy(dst[:, bass.ts(sb, SBK)], ps[:])
            return run

        def v_evict():
            vt = vt_pool.tile([128, SBK], F32, tag="vt", name="vt")
            nc.vector.tensor_copy(vt[:], state["v"][:])
            state["vt"] = vt

        def t_unit():
            tp = ps_wk.tile([128, SBK], F32, tag="work", name="ps_tp")
            for t in range(4):
                jt = sb * 4 + t
                tsl = bass.ts(t, 128)
                nc.tensor.transpose(tp[:, tsl], state["vt"][:, tsl], ident[:])
                tp_r = bass.AP(tensor=tp.tensor, offset=tp.offset + t * 128,
                               ap=[list(tp.ap[0]), [HD, 2], [1, HD]])
                nc.vector.tensor_copy(v_aug_r[:, jt, 0::2, :], tp_r)

        for which in ("q", "k", "v"):
            for lo in (0, 4):
                units.append(qk_unit(which, lo))
        units.append(v_evict)
        units.append(t_unit)
        return units

    def proj_units(qb, tail=False):
        """Output-projection partial for s-block qb -> DRAM (transposed).
        At the tail there is no attention left to hide behind, so borrow
        the (now idle) score-psum banks and the ScalarE for every other
        eviction to shorten the drain."""
        qsl = bass.ts(qb, SBK)
        units = []

        def ob_unit(ob):
            def run():
                if tail and ob % 2 == 0:
                    big = ps_sc.tile([128, 2 * SBK], F32, tag="sc",
                                     name="ps_po_sc")
                    po = big[:, 0:SBK]
                else:
                    po = ps_wk.tile([128, SBK], F32, tag="work",
                                    name="ps_po")[:]
                nc.tensor.matmul(po, lhsT=wo_sb[:, bass.ts(ob, 128)],
                                 rhs=attnT[:, qsl], start=True, stop=True)
                ot = ot_pool.tile([128, SBK], F32, tag="ot")
                if tail and ob % 2 == 1:
                    nc.scalar.activation(ot[:], po,
                                         mybir.ActivationFunctionType.Copy)
                else:
                    nc.vector.tensor_copy(ot[:], po)
                if tail and ob % 3 == 1:
                    nc.gpsimd.dma_start(out=outp[bass.ts(ob, 128), qsl],
                                        in_=ot[:])
                elif tail and ob % 3 == 2:
                    nc.scalar.dma_start(out=outp[bass.ts(ob, 128), qsl],
                                        in_=ot[:])
                else:
                    nc.sync.dma_start(out=outp[bass.ts(ob, 128), qsl],
                                      in_=ot[:])
            return run

        for ob in range(8):
            units.append(ob_unit(ob))
        return units

    scale = float(1.0 / np.sqrt(HD))

    def attention(qb, units):
        """Causal attention for query block qb (both heads), with `units`
        (phase1/proj closures) interleaved into the PE stream."""
        nj = 4 * (qb + 1)               # valid j128-blocks
        # diagonal strips first: their GpSimd mask latency hides under
        # the long tail of full-width blocks.
        order = list(range(nj - 4, nj)) + list(range(0, nj - 4))
        offs = {j: max(0, 128 * (j - (nj - 4))) for j in order}

        acc_A = ps_acc.tile([128, SBK], F32, tag="acc", name="acc_A")
        acc_B = ps_acc.tile([128, SBK], F32, tag="acc", name="acc_B")

        def emit_sc(j):
            off = offs[j]
            sc = ps_sc.tile([128, 2 * SBK], F32, tag="sc", name="sc")
            qa = qT_sb[0:64, qb * SBK + off:(qb + 1) * SBK]
            qb_ap = qT_sb[64:128, qb * SBK + off:(qb + 1) * SBK]
            nc.tensor.matmul(sc[:, off:SBK],
                             lhsT=kT_sb[0:64, bass.ts(j, JBK)],
                             rhs=qa, start=True, stop=True)
            nc.tensor.matmul(sc[:, SBK + off:2 * SBK],
                             lhsT=kT_sb[64:128, bass.ts(j, JBK)],
                             rhs=qb_ap, start=True, stop=True)
            return sc

        def emit_exp(j, sc):
            off = offs[j]
            pt = pt_pool.tile([128, 2 * SBK], BF16, tag="pt", name="pt")
            if off == 0:
                nc.scalar.activation(pt[:], sc[:],
                                     mybir.ActivationFunctionType.Exp,
                                     scale=scale)
            else:
                w = SBK - off
                sc2 = bass.AP(tensor=sc.tensor, offset=sc.offset + off,
                              ap=[list(sc.ap[0]), [SBK, 2], [1, w]])
                pt2 = bass.AP(tensor=pt.tensor, offset=pt.offset + off,
                              ap=[list(pt.ap[0]), [SBK, 2], [1, w]])
                nc.scalar.activation(pt2, sc2,
                                     mybir.ActivationFunctionType.Exp,
                                     scale=scale)
            return pt

        def emit_mask(j, pt):
            # bf16 SBUF operands hit the DVE 2x mode (~190ns); also keeps
            # the Pool engine free for its software-DGE duties.
            r = j - (nj - 4)
            dlo = 128 * r
            nc.vector.tensor_mul(pt[:, dlo:dlo + 128],
                                 pt[:, dlo:dlo + 128], mask_sb[:])
            nc.vector.tensor_mul(pt[:, SBK + dlo:SBK + dlo + 128],
                                 pt[:, SBK + dlo:SBK + dlo + 128],
                                 mask_sb[:])

        def emit_pv(idx, j, pt):
            off = offs[j]
            st, sp = idx == 0, idx == nj - 1
            vb = j * VW
            nc.tensor.matmul(acc_A[:, off:SBK],
                             lhsT=v_aug[:, vb:vb + 128],
                             rhs=pt[:, off:SBK], start=st, stop=sp)
            nc.tensor.matmul(acc_B[:, off:SBK],
                             lhsT=v_aug[:, vb + HD:vb + VW],
                             rhs=pt[:, SBK + off:2 * SBK],
                             start=st, stop=sp)

        # Lookahead-2 software pipeline: pv(j) is emitted one full
        # iteration after exp(j), so its semaphore fired a whole period
        # earlier and the PE queue never head-of-line blocks.  Stalls
        # reset the tensor engine's pstate ramp (1.2 -> 2.4 GHz needs
        # ~3us of continuous execution), so this is what keeps the PE
        # at full clock.
        emitted = 0
        sc = emit_sc(order[0])
        pts = {}
        for idx, j in enumerate(order):
            pts[idx] = emit_exp(j, sc)
            if j >= nj - 4:
                emit_mask(j, pts[idx])
            if idx + 1 < nj:
                sc = emit_sc(order[idx + 1])
            want = ((idx + 1) * len(units)) // (nj + 1)
            while emitted < want:
                units[emitted]()
                emitted += 1
            if idx >= 1:
                emit_pv(idx - 1, order[idx - 1], pts.pop(idx - 1))
        while emitted < len(units):
            units[emitted]()
            emitted += 1
        emit_pv(nj - 1, order[nj - 1], pts.pop(nj - 1))
        return acc_A, acc_B

    def normalize(qb, acc_A, acc_B):
        # head A: out rows 0:64, l rows 64:128; head B flipped.
        # 1/l as exp(-ln(l)) on ScalarE (Exp+Ln share one pinned table
        # set so no ACT_TABLE_LOADs). DVE InstReciprocal costs 3.4us/op
        # on hardware; DVE divide and reciprocal_approx_fast fail on
        # hardware (sim-only).
        qsl = bass.ts(qb, SBK)
        lnl_a = lv_pool.tile([64, SBK], F32, tag="lv")
        nc.scalar.activation(lnl_a[:], acc_A[HD:2 * HD, :],
                             mybir.ActivationFunctionType.Ln)
        linv_a = lv_pool.tile([64, SBK], F32, tag="lv2")
        nc.scalar.activation(linv_a[:], lnl_a[:],
                             mybir.ActivationFunctionType.Exp, scale=-1.0)
        nc.vector.tensor_mul(attnT[0:64, qsl], acc_A[0:HD, :], linv_a[:])
        lnl_b = lv_pool.tile([64, SBK], F32, tag="lv")
        nc.scalar.activation(lnl_b[:], acc_B[0:HD, :],
                             mybir.ActivationFunctionType.Ln)
        linv_b = lv_pool.tile([64, SBK], F32, tag="lv2")
        nc.scalar.activation(linv_b[:], lnl_b[:],
                             mybir.ActivationFunctionType.Exp, scale=-1.0)
        nc.vector.tensor_mul(attnT[64:128, qsl], acc_B[HD:2 * HD, :],
                             linv_b[:])

    # prologue: phase1(0) un-interleaved, then the qb loop with
    # phase1(qb+1) + proj(qb-1) threaded into attention(qb)'s j-loop.
    # x loads are prefetched two blocks ahead so phase1 units never wait.
    # xb(1) is split across both DMA queues and slotted between the
    # startup-critical const loads.
    xb_handles = {0: phase1_dma(0)}
    xbt1 = xb_pool.tile([128, 8 * SBK], BF16, tag="xb", name="xbt1")
    xb1_r = xbt1[:].rearrange("p (dt s) -> p dt s", dt=8)
    sl1 = slice(SBK, 2 * SBK)
    nc.sync.dma_start(out=xb1_r[:, :4, :], in_=xb_r_dram[:, :4, sl1])
    nc.sync.dma_start(out=wv_sb[:], in_=wv[:])
    nc.gpsimd.dma_start(out=xb1_r[:, 4:, :], in_=xb_r_dram[:, 4:, sl1])
    nc.sync.dma_start(out=mask_sb[:], in_=dmask[:])
    nc.sync.dma_start(out=v_aug_r[:, :, 1, :], in_=ones_bcast)
    xb_handles[1] = xb1_r
    for u in phase1_units(0, xb_handles[0]):
        u()
    _late_consts()
    for qb in range(NSB):
        if qb + 2 < NSB:
            xb_handles[qb + 2] = phase1_dma(qb + 2)
        units = []
        if qb + 1 < NSB:
            units += phase1_units(qb + 1, xb_handles[qb + 1])
        if qb > 0:
            units += proj_units(qb - 1)
        accs = attention(qb, units)
        normalize(qb, *accs)
    for u in proj_units(NSB - 1, tail=True):
        u()


def _host_prep(x, Wq, Wk, Wv, Wo):
    xf = np.ascontiguousarray(x.reshape(S, D)).astype(np.float32)
    xT = xf.T  # [D, S]
    xb = np.ascontiguousarray(
        xT.reshape(8, 128, S).transpose(1, 0, 2)
    ).astype(NP_BF16).reshape(128, 8 * S)

    jj = np.arange(JBK)[:, None]
    qq = np.arange(JBK)[None, :]
    dmask = (jj <= qq).astype(NP_BF16)
    ones_arr = np.ones((128, HD), dtype=NP_BF16)

    def w_prep(W_local):
        # [p, dt, m] = W_local[m, dt*128+p]
        A = W_local.T.reshape(8, 128, M).transpose(1, 0, 2)
        return np.ascontiguousarray(A).astype(NP_BF16).reshape(128, -1)

    in_maps = []
    for c in range(NCORES):
        sl = slice(c * M, (c + 1) * M)
        in_maps.append({
            "xb": xb,
            "wq": w_prep(np.asarray(Wq[sl, :], np.float32)),
            "wk": w_prep(np.asarray(Wk[sl, :], np.float32)),
            "wv": w_prep(np.asarray(Wv[sl, :], np.float32)),
            "wo": np.ascontiguousarray(Wo[:, sl].T).astype(np.float32),
            "ones": ones_arr,
            "dmask": dmask,
        })
    return in_maps


def _run(inputs, trace=False):
    x = np.asarray(inputs["x"], dtype=np.float32)
    Wq = np.asarray(inputs["Wq"], dtype=np.float32)
    Wk = np.asarray(inputs["Wk"], dtype=np.float32)
    Wv = np.asarray(inputs["Wv"], dtype=np.float32)
    Wo = np.asarray(inputs["Wo"], dtype=np.float32)

    if "nc" not in _CACHE:
        _CACHE["nc"] = _build_nc()
    nc = _CACHE["nc"]

    in_maps = _host_prep(x, Wq, Wk, Wv, Wo)
    res = bass_utils.run_bass_kernel_spmd(
        nc, in_maps, core_ids=list(range(NCORES)), trace=trace)
    partial = np.zeros((D, S), dtype=np.float64)
    for c in range(NCORES):
        partial += res.results[c]["outp"].astype(np.float64)
    out = partial.T.astype(np.float32).reshape(B, S, D)
    return out, res


def kernel(x, mask, Wq, Wk, Wv, Wo):
    mask = np.asarray(mask)
    causal = np.tril(np.ones((S, S), dtype=bool))
    if mask.reshape(S, S).shape == causal.shape and bool(
            np.array_equal(mask.reshape(S, S), causal)):
        out, _ = _run({"x": x, "Wq": Wq, "Wk": Wk, "Wv": Wv, "Wo": Wo})
        return out
    # safety net for a non-causal mask: exact numpy fallback
    return _numpy_ref(np.asarray(x, np.float32), mask,
                      np.asarray(Wq, np.float32), np.asarray(Wk, np.float32),
                      np.asarray(Wv, np.float32), np.asarray(Wo, np.float32))


def _numpy_ref(x, mask, Wq, Wk, Wv, Wo):
    xf = x.reshape(S, D)
    q = xf @ Wq.T
    k = xf @ Wk.T
    v = xf @ Wv.T
    m2 = mask.reshape(S, S)
    o = np.empty((S, D), dtype=np.float32)
    for h in range(H):
        hs = slice(h * HD, (h + 1) * HD)
        sc = (q[:, hs] @ k[:, hs].T) / np.sqrt(np.float32(HD))
        sc = np.where(m2, sc, np.float32(-1e9))
        sc -= sc.max(axis=-1, keepdims=True)
        p = np.exp(sc)
        p /= p.sum(axis=-1, keepdims=True)
        o[:, hs] = p @ v[:, hs]
    return (o @ Wo.T).astype(np.float32).reshape(B, S, D)


# revision 41
# speedup vs baseline: 1.3575x; 1.0049x over previous
"""Multi-head causal attention (B=1, S=4096, D=1024, H=16, HD=64) on 8
Trainium2 NeuronCores.

Sharding: head-parallel - 16 heads / 8 cores = 2 heads per core (one
128-channel slice of the QKV/output projections per core).

Per-core pipeline (contraction dims on SBUF partitions; softmax exp
reads PSUM directly; all attention matmuls in bf16 - fp8 was measured
to put ~5% relative error on the output because softmax-weight noise
is not damped by the value average):

  phase 1 (interleaved into the attention j-loop as filler PE units):
    qT/kT [128, S] bf16 via wq-stationary matmuls (outputs transposed),
    V via vT matmuls + PE transpose per 128-tile, stored per j-tile as
    [V_A | ones | V_B] in bf16. x loads are prefetched two s-blocks
    ahead on alternating DMA queues.
  phase 2 flash-style attention, no max-subtraction (scores ~ N(0,1)):
    scoresT psum[j, A|B] = kT_j.T @ qT_q (2 heads, K=64 each)
    PT = exp(scores/8): ScalarE reads PSUM, writes bf16 SBUF
    causal: strictly-upper j-blocks skipped, partial-width operands on
      diagonal blocks, one [128,128] triangle mask multiply (DVE 2x);
      diagonal blocks processed FIRST so their mask latency hides
      under the long run of full-width blocks.
    acc += [V | ones].T @ PT (bf16, rows 64:128 = softmax denominator)
    The j-loop is software-pipelined with LOOKAHEAD 2: attnV(j) is
      emitted one full iteration after exp(j), so its semaphore fired a
      period earlier and the in-order PE queue never stalls - stalls
      reset the tensor engine's pstate ramp (1.2 -> 2.4 GHz needs ~3us
      of continuous execution), which is worth ~2x on every matmul.
  normalize: 1/l as exp(-ln(l)) on ScalarE (Exp+Ln pinned to one ACT
    table set; DVE InstReciprocal costs 3.4us/op on hardware and both
    DVE divide and reciprocal_approx_fast are simulator-only),
    attnT = acc * (1/l) on DVE.
  phase 3 output projection partial (f32r): partialT[o, s], written
    transposed [1024, 4096] per core; host sums the 8 partials. The
    final block's projection drains through both score-psum banks with
    evictions alternating DVE/ScalarE and stores across 3 DMA queues.

Measured on HW: 227.5us, rel err 5.1e-3 (baseline: 307-317us, 3.5e-4).
"""

import os
import sys

import numpy as np

for _p in ("/opt/trn_rl_repo", "/root/.axon_site/_ro/trn_rl_repo"):
    if os.path.isdir(_p) and _p not in sys.path:
        sys.path.insert(0, _p)

from contextlib import ExitStack

import ml_dtypes

import concourse.bass as bass
import concourse.tile as tile
from concourse import bacc, bass_utils, mybir
from concourse.masks import make_identity
import concourse.hw_specs as _hw_specs
import functools as _functools

# Pin Exp/Ln to the one activation-table set containing both, so the
# softmax exp and the exp(-ln(l)) normalization never ping-pong
# ACT_TABLE_LOADs. Only the *selection* map is filtered; set order
# (= act_func_set_id) is unchanged. (Same mechanism as the original
# baseline kernel.)
_orig_get_tables = _hw_specs.get_activation_tables


@_functools.cache
def _pinned_tables(arch):
    t = dict(_orig_get_tables(arch))
    strip = {mybir.ActivationFunctionType.Exp, mybir.ActivationFunctionType.Ln}
    for name in t:
        if name != "natural_log_exp_and_others":
            t[name] = t[name] - strip
    return t


_hw_specs.get_activation_tables = _pinned_tables
bacc.get_activation_tables = _pinned_tables

# Problem shape (hardcoded per the harness contract).
B, S, D, H = 1, 4096, 1024, 16
HD = D // H          # 64
NCORES = 8
HPC = H // NCORES    # 2 heads per core
M = HPC * HD         # 128 channels per core
SBK = 512            # s/q block size
NSB = S // SBK       # 8
JBK = 128            # j (key) block size
NJT = S // JBK       # 32
VW = 3 * HD          # v_aug row width per j-tile: [V_A | ones | V_B]

F32 = mybir.dt.float32
F32R = mybir.dt.float32r
BF16 = mybir.dt.bfloat16

NP_BF16 = ml_dtypes.bfloat16

_CACHE = {}


def _build_nc():
    """Build + compile the per-core Bass program (identical on all cores)."""
    nc = bacc.Bacc("TRN2", target_bir_lowering=False, debug=False,
                   num_devices=NCORES)

    # bf16 x, pre-transposed: [p, dt, s] with d = dt*128+p
    xb = nc.dram_tensor("xb", [128, 8 * S], BF16, kind="ExternalInput").ap()
    # bf16 weights, [p, dt, m] = W[m_local, dt*128+p]
    wq = nc.dram_tensor("wq", [128, 8 * M], BF16, kind="ExternalInput").ap()
    wk = nc.dram_tensor("wk", [128, 8 * M], BF16, kind="ExternalInput").ap()
    wv = nc.dram_tensor("wv", [128, 8 * M], BF16, kind="ExternalInput").ap()
    wo = nc.dram_tensor("wo", [M, D], F32R, kind="ExternalInput").ap()
    ones = nc.dram_tensor("ones", [128, HD], BF16, kind="ExternalInput").ap()
    dmask = nc.dram_tensor("dmask", [JBK, JBK], BF16,
                           kind="ExternalInput").ap()
    outp = nc.dram_tensor("outp", [D, S], F32, kind="ExternalOutput").ap()

    with tile.TileContext(nc) as tc:
        with ExitStack() as ctx:
            _emit(ctx, tc, nc, xb, wq, wk, wv, wo, ones, dmask, outp)
    nc.compile()
    return nc


def _emit(ctx, tc, nc, xb, wq, wk, wv, wo, ones, dmask, outp):
    const = ctx.enter_context(tc.tile_pool(name="const", bufs=1))
    persist = ctx.enter_context(tc.tile_pool(name="persist", bufs=1))
    xb_pool = ctx.enter_context(tc.tile_pool(name="xb", bufs=3))
    vt_pool = ctx.enter_context(tc.tile_pool(name="vt", bufs=2))
    pt_pool = ctx.enter_context(tc.tile_pool(name="pt", bufs=4))
    ot_pool = ctx.enter_context(tc.tile_pool(name="ot", bufs=8))
    lv_pool = ctx.enter_context(tc.tile_pool(name="lv", bufs=2))
    # PSUM: 16 KB/partition = 8 banks. sc 2x(2 banks) + acc 2x1 + work 2x1.
    ps_sc = ctx.enter_context(tc.tile_pool(name="ps_sc", bufs=2, space="PSUM"))
    ps_acc = ctx.enter_context(tc.tile_pool(name="ps_acc", bufs=2,
                                            space="PSUM"))
    ps_wk = ctx.enter_context(tc.tile_pool(name="ps_wk", bufs=2, space="PSUM"))

    # ---- constants ----
    ident = const.tile([128, 128], F32)
    make_identity(nc, ident)
    # warm the ScalarE Exp table at t=0 so the first real softmax exp
    # does not pay the 1.3us ACT_TABLE_LOAD on the critical path.
    warm = const.tile([64, 4], F32)
    nc.scalar.activation(warm[:], ident[0:64, 0:4],
                         mybir.ActivationFunctionType.Exp)
    wq_sb = const.tile([128, 8 * M], BF16)
    wk_sb = const.tile([128, 8 * M], BF16)
    wv_sb = const.tile([128, 8 * M], BF16)
    wo_sb = const.tile([128, D], F32R)
    mask_sb = const.tile([128, JBK], BF16)
    # startup-critical sync-queue order: wq, wk (phase1(0) q/k), then
    # the first half of xb(1) slots in before wv; mask/ones before the
    # first exp/pv. xb(0) flows on the gpsimd queue in parallel.
    nc.sync.dma_start(out=wq_sb[:], in_=wq[:])
    nc.sync.dma_start(out=wk_sb[:], in_=wk[:])

    def _late_consts():
        for c in range(4):
            nc.sync.dma_start(out=wo_sb[:, bass.ts(c, 256)],
                              in_=wo[:, bass.ts(c, 256)])

    wq_r = wq_sb[:].rearrange("p (dt m) -> p dt m", dt=8)
    wk_r = wk_sb[:].rearrange("p (dt m) -> p dt m", dt=8)
    wv_r = wv_sb[:].rearrange("p (dt m) -> p dt m", dt=8)

    # ---- persistent SBUF ----
    qT_sb = persist.tile([128, S], BF16)
    kT_sb = persist.tile([128, S], BF16)
    v_aug = persist.tile([128, NJT * VW], BF16)
    attnT = persist.tile([128, S], F32R)

    # per j-tile layout [V_A | ones | V_B]: head A reads cols 0:128
    # ([V_A | 1] -> acc rows 64:128 = denominator l), head B reads cols
    # 64:192 ([1 | V_B] -> acc rows 0:64 = l). ones filled by one
    # broadcast DMA (step-0 middle dim on input).
    v_aug_r = v_aug[:].rearrange("p (t c w) -> p t c w", c=3, w=HD)
    ones_bcast = bass.AP(
        tensor=ones.tensor, offset=0,
        ap=[[HD, 128], [0, NJT], [1, HD]])

    xb_r_dram = xb.rearrange("p (dt s) -> p dt s", dt=8)

    def phase1_dma(sb):
        """x loads for s-block sb (issued one attention block early)."""
        xbt = xb_pool.tile([128, 8 * SBK], BF16, tag="xb", name="xbt")
        xb_r = xbt[:].rearrange("p (dt s) -> p dt s", dt=8)
        sl = slice(sb * SBK, (sb + 1) * SBK)
        # alternate DMA queues per s-block so consecutive prefetches do
        # not serialize behind each other on one DGE; sb=2 rides the
        # otherwise-idle ScalarE DGE during the congested startup.
        if sb == 2:
            eng = nc.scalar
        else:
            eng = nc.gpsimd if sb % 2 == 0 else nc.sync
        if sb == 0:
            # dtile-major chunks so the first q matmuls start after ~1
            # chunk instead of the full 8 KB/partition load.
            for d0 in range(0, 8, 2):
                eng.dma_start(out=xb_r[:, d0:d0 + 2, :],
                              in_=xb_r_dram[:, d0:d0 + 2, sl])
        else:
            eng.dma_start(out=xb_r[:, :4, :], in_=xb_r_dram[:, :4, sl])
            eng.dma_start(out=xb_r[:, 4:, :], in_=xb_r_dram[:, 4:, sl])
        return xb_r

    def phase1_units(sb, xb_r):
        """QKV projections for s-block sb, as PE work units."""
        units = []
        state = {}

        def qk_unit(which, lo):
            def run():
                w_r = {"q": wq_r, "k": wk_r, "v": wv_r}[which]
                if lo == 0:
                    state[which] = ps_wk.tile([128, SBK], F32, tag="work",
                                              name=f"ps_{which}")
                ps = state[which]
                for dt in range(lo, lo + 4):
                    nc.tensor.matmul(ps[:], lhsT=w_r[:, dt, :],
                                     rhs=xb_r[:, dt, :],
                                     start=dt == 0, stop=dt == 7)
                if lo == 4 and which != "v":
                    dst = qT_sb if which == "q" else kT_sb
                    nc.vector.tensor_copy(dst[:, bass.ts(sb, SBK)], ps[:])
            return run

        def v_evict():
            vt = vt_pool.tile([128, SBK], F32, tag="vt", name="vt")
            nc.vector.tensor_copy(vt[:], state["v"][:])
            state["vt"] = vt

        def t_unit():
            tp = ps_wk.tile([128, SBK], F32, tag="work", name="ps_tp")
            for t in range(4):
                jt = sb * 4 + t
                tsl = bass.ts(t, 128)
                nc.tensor.transpose(tp[:, tsl], state["vt"][:, tsl], ident[:])
                tp_r = bass.AP(tensor=tp.tensor, offset=tp.offset + t * 128,
                               ap=[list(tp.ap[0]), [HD, 2], [1, HD]])
                nc.vector.tensor_copy(v_aug_r[:, jt, 0::2, :], tp_r)

        for which in ("q", "k", "v"):
            for lo in (0, 4):
                units.append(qk_unit(which, lo))
        units.append(v_evict)
        units.append(t_unit)
        return units

    def proj_units(qb, tail=False):
        """Output-projection partial for s-block qb -> DRAM (transposed).
        At the tail there is no attention left to hide behind, so borrow
        the (now idle) score-psum banks and the ScalarE for every other
        eviction to shorten the drain."""
        qsl = bass.ts(qb, SBK)
        units = []

        def ob_unit(ob):
            def run():
                if tail and ob % 2 == 0:
                    big = ps_sc.tile([128, 2 * SBK], F32, tag="sc",
                                     name="ps_po_sc")
                    po = big[:, 0:SBK]
                else:
                    po = ps_wk.tile([128, SBK], F32, tag="work",
                                    name="ps_po")[:]
                nc.tensor.matmul(po, lhsT=wo_sb[:, bass.ts(ob, 128)],
                                 rhs=attnT[:, qsl], start=True, stop=True)
                ot = ot_pool.tile([128, SBK], F32, tag="ot")
                if tail and ob % 2 == 1:
                    nc.scalar.activation(ot[:], po,
                                         mybir.ActivationFunctionType.Copy)
                else:
                    nc.vector.tensor_copy(ot[:], po)
                if tail and ob % 3 == 1:
                    nc.gpsimd.dma_start(out=outp[bass.ts(ob, 128), qsl],
                                        in_=ot[:])
                elif tail and ob % 3 == 2:
                    nc.scalar.dma_start(out=outp[bass.ts(ob, 128), qsl],
                                        in_=ot[:])
                else:
                    nc.sync.dma_start(out=outp[bass.ts(ob, 128), qsl],
                                      in_=ot[:])
            return run

        for ob in range(8):
            units.append(ob_unit(ob))
        return units

    scale = float(1.0 / np.sqrt(HD))

    def attention(qb, units):
        """Causal attention for query block qb (both heads), with `units`
        (phase1/proj closures) interleaved into the PE stream."""
        nj = 4 * (qb + 1)               # valid j128-blocks
        # diagonal strips first: their GpSimd mask latency hides under
        # the long tail of full-width blocks.
        order = list(range(nj - 4, nj)) + list(range(0, nj - 4))
        offs = {j: max(0, 128 * (j - (nj - 4))) for j in order}

        acc_A = ps_acc.tile([128, SBK], F32, tag="acc", name="acc_A")
        acc_B = ps_acc.tile([128, SBK], F32, tag="acc", name="acc_B")

        def emit_sc(j):
            off = offs[j]
            sc = ps_sc.tile([128, 2 * SBK], F32, tag="sc", name="sc")
            qa = qT_sb[0:64, qb * SBK + off:(qb + 1) * SBK]
            qb_ap = qT_sb[64:128, qb * SBK + off:(qb + 1) * SBK]
            nc.tensor.matmul(sc[:, off:SBK],
                             lhsT=kT_sb[0:64, bass.ts(j, JBK)],
                             rhs=qa, start=True, stop=True)
            nc.tensor.matmul(sc[:, SBK + off:2 * SBK],
                             lhsT=kT_sb[64:128, bass.ts(j, JBK)],
                             rhs=qb_ap, start=True, stop=True)
            return sc

        def emit_exp(j, sc):
            off = offs[j]
            pt = pt_pool.tile([128, 2 * SBK], BF16, tag="pt", name="pt")
            if off == 0:
                nc.scalar.activation(pt[:], sc[:],
                                     mybir.ActivationFunctionType.Exp,
                                     scale=scale)
            else:
                w = SBK - off
                sc2 = bass.AP(tensor=sc.tensor, offset=sc.offset + off,
                              ap=[list(sc.ap[0]), [SBK, 2], [1, w]])
                pt2 = bass.AP(tensor=pt.tensor, offset=pt.offset + off,
                              ap=[list(pt.ap[0]), [SBK, 2], [1, w]])
                nc.scalar.activation(pt2, sc2,
                                     mybir.ActivationFunctionType.Exp,
                                     scale=scale)
            return pt

        def emit_mask(j, pt):
            # bf16 SBUF operands hit the DVE 2x mode (~190ns); also keeps
            # the Pool engine free for its software-DGE duties.
            r = j - (nj - 4)
            dlo = 128 * r
            nc.vector.tensor_mul(pt[:, dlo:dlo + 128],
                                 pt[:, dlo:dlo + 128], mask_sb[:])
            nc.vector.tensor_mul(pt[:, SBK + dlo:SBK + dlo + 128],
                                 pt[:, SBK + dlo:SBK + dlo + 128],
                                 mask_sb[:])

        def emit_pv(idx, j, pt):
            off = offs[j]
            st, sp = idx == 0, idx == nj - 1
            vb = j * VW
            nc.tensor.matmul(acc_A[:, off:SBK],
                             lhsT=v_aug[:, vb:vb + 128],
                             rhs=pt[:, off:SBK], start=st, stop=sp)
            nc.tensor.matmul(acc_B[:, off:SBK],
                             lhsT=v_aug[:, vb + HD:vb + VW],
                             rhs=pt[:, SBK + off:2 * SBK],
                             start=st, stop=sp)

        # Lookahead-2 software pipeline: pv(j) is emitted one full
        # iteration after exp(j), so its semaphore fired a whole period
        # earlier and the PE queue never head-of-line blocks.  Stalls
        # reset the tensor engine's pstate ramp (1.2 -> 2.4 GHz needs
        # ~3us of continuous execution), so this is what keeps the PE
        # at full clock.
        emitted = 0
        sc = emit_sc(order[0])
        pts = {}
        for idx, j in enumerate(order):
            pts[idx] = emit_exp(j, sc)
            if j >= nj - 4:
                emit_mask(j, pts[idx])
            if idx + 1 < nj:
                sc = emit_sc(order[idx + 1])
            want = ((idx + 1) * len(units)) // (nj + 1)
            while emitted < want:
                units[emitted]()
                emitted += 1
            if idx >= 1:
                emit_pv(idx - 1, order[idx - 1], pts.pop(idx - 1))
        while emitted < len(units):
            units[emitted]()
            emitted += 1
        emit_pv(nj - 1, order[nj - 1], pts.pop(nj - 1))
        return acc_A, acc_B

    def normalize(qb, acc_A, acc_B):
        # head A: out rows 0:64, l rows 64:128; head B flipped.
        # 1/l as exp(-ln(l)) on ScalarE (Exp+Ln share one pinned table
        # set so no ACT_TABLE_LOADs). DVE InstReciprocal costs 3.4us/op
        # on hardware; DVE divide and reciprocal_approx_fast fail on
        # hardware (sim-only).
        qsl = bass.ts(qb, SBK)
        lnl_a = lv_pool.tile([64, SBK], F32, tag="lv")
        nc.scalar.activation(lnl_a[:], acc_A[HD:2 * HD, :],
                             mybir.ActivationFunctionType.Ln)
        linv_a = lv_pool.tile([64, SBK], F32, tag="lv2")
        nc.scalar.activation(linv_a[:], lnl_a[:],
                             mybir.ActivationFunctionType.Exp, scale=-1.0)
        nc.vector.tensor_mul(attnT[0:64, qsl], acc_A[0:HD, :], linv_a[:])
        lnl_b = lv_pool.tile([64, SBK], F32, tag="lv")
        nc.scalar.activation(lnl_b[:], acc_B[0:HD, :],
                             mybir.ActivationFunctionType.Ln)
        linv_b = lv_pool.tile([64, SBK], F32, tag="lv2")
        nc.scalar.activation(linv_b[:], lnl_b[:],
                             mybir.ActivationFunctionType.Exp, scale=-1.0)
        nc.vector.tensor_mul(attnT[64:128, qsl], acc_B[HD:2 * HD, :],
                             linv_b[:])

    # prologue: phase1(0) un-interleaved, then the qb loop with
    # phase1(qb+1) + proj(qb-1) threaded into attention(qb)'s j-loop.
    # x loads are prefetched two blocks ahead so phase1 units never wait.
    # xb(1) is split across both DMA queues and slotted between the
    # startup-critical const loads.
    xb_handles = {0: phase1_dma(0)}
    xbt1 = xb_pool.tile([128, 8 * SBK], BF16, tag="xb", name="xbt1")
    xb1_r = xbt1[:].rearrange("p (dt s) -> p dt s", dt=8)
    sl1 = slice(SBK, 2 * SBK)
    nc.sync.dma_start(out=xb1_r[:, :4, :], in_=xb_r_dram[:, :4, sl1])
    nc.sync.dma_start(out=wv_sb[:], in_=wv[:])
    nc.gpsimd.dma_start(out=xb1_r[:, 4:, :], in_=xb_r_dram[:, 4:, sl1])
    # mask/ones go through the ScalarE DGE - a third queue that is idle
    # until the first exp fires at ~6us.
    nc.scalar.dma_start(out=mask_sb[:], in_=dmask[:])
    nc.scalar.dma_start(out=v_aug_r[:, :, 1, :], in_=ones_bcast)
    xb_handles[1] = xb1_r
    for u in phase1_units(0, xb_handles[0]):
        u()
    _late_consts()
    for qb in range(NSB):
        if qb + 2 < NSB:
            xb_handles[qb + 2] = phase1_dma(qb + 2)
        # projections are deferred TWO blocks (proj(qb-2) inside
        # attention(qb)) so the last attention block - which has no
        # phase1 work left - still gets 16 PE filler units and stays
        # PE-bound; qb7 absorbs proj(5) and proj(6), only proj(7)
        # remains for the tail.
        units = []
        if qb + 1 < NSB:
            units += phase1_units(qb + 1, xb_handles[qb + 1])
        if 2 <= qb < NSB - 1:
            units += proj_units(qb - 2)
        elif qb == NSB - 1:
            units += proj_units(qb - 2) + proj_units(qb - 1)
        accs = attention(qb, units)
        normalize(qb, *accs)
    for u in proj_units(NSB - 1, tail=True):
        u()


def _host_prep(x, Wq, Wk, Wv, Wo):
    xf = np.ascontiguousarray(x.reshape(S, D)).astype(np.float32)
    xT = xf.T  # [D, S]
    xb = np.ascontiguousarray(
        xT.reshape(8, 128, S).transpose(1, 0, 2)
    ).astype(NP_BF16).reshape(128, 8 * S)

    jj = np.arange(JBK)[:, None]
    qq = np.arange(JBK)[None, :]
    dmask = (jj <= qq).astype(NP_BF16)
    ones_arr = np.ones((128, HD), dtype=NP_BF16)

    def w_prep(W_local):
        # [p, dt, m] = W_local[m, dt*128+p]
        A = W_local.T.reshape(8, 128, M).transpose(1, 0, 2)
        return np.ascontiguousarray(A).astype(NP_BF16).reshape(128, -1)

    in_maps = []
    for c in range(NCORES):
        sl = slice(c * M, (c + 1) * M)
        in_maps.append({
            "xb": xb,
            "wq": w_prep(np.asarray(Wq[sl, :], np.float32)),
            "wk": w_prep(np.asarray(Wk[sl, :], np.float32)),
            "wv": w_prep(np.asarray(Wv[sl, :], np.float32)),
            "wo": np.ascontiguousarray(Wo[:, sl].T).astype(np.float32),
            "ones": ones_arr,
            "dmask": dmask,
        })
    return in_maps


def _run(inputs, trace=False):
    x = np.asarray(inputs["x"], dtype=np.float32)
    Wq = np.asarray(inputs["Wq"], dtype=np.float32)
    Wk = np.asarray(inputs["Wk"], dtype=np.float32)
    Wv = np.asarray(inputs["Wv"], dtype=np.float32)
    Wo = np.asarray(inputs["Wo"], dtype=np.float32)

    if "nc" not in _CACHE:
        _CACHE["nc"] = _build_nc()
    nc = _CACHE["nc"]

    in_maps = _host_prep(x, Wq, Wk, Wv, Wo)
    res = bass_utils.run_bass_kernel_spmd(
        nc, in_maps, core_ids=list(range(NCORES)), trace=trace)
    partial = np.zeros((D, S), dtype=np.float64)
    for c in range(NCORES):
        partial += res.results[c]["outp"].astype(np.float64)
    out = partial.T.astype(np.float32).reshape(B, S, D)
    return out, res


def kernel(x, mask, Wq, Wk, Wv, Wo):
    mask = np.asarray(mask)
    causal = np.tril(np.ones((S, S), dtype=bool))
    if mask.reshape(S, S).shape == causal.shape and bool(
            np.array_equal(mask.reshape(S, S), causal)):
        out, _ = _run({"x": x, "Wq": Wq, "Wk": Wk, "Wv": Wv, "Wo": Wo})
        return out
    # safety net for a non-causal mask: exact numpy fallback
    return _numpy_ref(np.asarray(x, np.float32), mask,
                      np.asarray(Wq, np.float32), np.asarray(Wk, np.float32),
                      np.asarray(Wv, np.float32), np.asarray(Wo, np.float32))


def _numpy_ref(x, mask, Wq, Wk, Wv, Wo):
    xf = x.reshape(S, D)
    q = xf @ Wq.T
    k = xf @ Wk.T
    v = xf @ Wv.T
    m2 = mask.reshape(S, S)
    o = np.empty((S, D), dtype=np.float32)
    for h in range(H):
        hs = slice(h * HD, (h + 1) * HD)
        sc = (q[:, hs] @ k[:, hs].T) / np.sqrt(np.float32(HD))
        sc = np.where(m2, sc, np.float32(-1e9))
        sc -= sc.max(axis=-1, keepdims=True)
        p = np.exp(sc)
        p /= p.sum(axis=-1, keepdims=True)
        o[:, hs] = p @ v[:, hs]
    return (o @ Wo.T).astype(np.float32).reshape(B, S, D)
